# revision 32
# baseline (speedup 1.0000x reference)
"""Trainium2 Bass kernel for modality-routed (CogVLM-style) attention, v3.

Contract: kernel(**inputs) takes FULL unsharded numpy inputs (as produced by
the reference's setup_inputs) and returns the FULL [2048, 4096] fp32 output.

Sharding: tensor-parallel over heads. Core r owns heads 4r..4r+3:
  - qkv weights column-sharded; q/k computed in transposed [dim, token]
    orientation via stationary weight tiles; v computed the same way then
    flipped to natural [token, dim] per 128-token tile with DMA transposes.
  - dense weights row-sharded; each core emits a partial output in transposed
    [4096, 2048] orientation, summed + transposed on the host.

v3 structural changes vs v2 (driven by the 880us trace):
  - attention is emitted at keytile granularity and INTERLEAVED between the
    next segment's QKV half-m-blocks, so exp (ACT) latency hides behind QKV
    matmuls instead of stalling the PE (attn windows were 45% PE-idle).
  - diagonal keytiles use restricted query widths (512/384/256/128) plus a
    single 128x128 triangular mask, cutting masked-out score/PV/exp work.
  - softmax normalization: ones-matmul partition sum -> reciprocal_approx_fast
    (vector.reciprocal was 3.3us per call) -> gpsimd partition_broadcast
    (replaces a DRAM bounce round-trip per head).
  - seg0's 64-token vision sliver shares LDWEIGHTS with the 512-token chunk
    (kt-major loop, two PSUM accumulators) instead of paying a LDW-bound pass.
  - DMAs are batched (1 per weight tile / segment / output row-block) and
    split across both HWDGE rings (plain on sync, transposes on scalar);
    the sync ring carried 496 serialized DMAs before.
  - dense runs chunk-major in two passes: tokens 0..1536 interleave with the
    last attention chunk, tokens 1536..2048 follow it; dense weights stream
    in 4-o-group tiles and output rows batch into 2 DMAs per row-block.
"""

import sys

import numpy as np

if "/opt/trn_rl_repo" not in sys.path:
    sys.path.insert(0, "/opt/trn_rl_repo")

import concourse.bass as bass  # noqa: E402,F401
import concourse.tile as tile  # noqa: E402
from concourse import bacc, mybir  # noqa: E402
from concourse.bass_utils import run_bass_kernel_spmd  # noqa: E402

S = 2048
HID = 4096
H = 32
D = 128
NCORES = 8
HPC = H // NCORES          # heads per core = 4
NV = 576                   # vision tokens occupy rows [0, NV)
NKT = HID // 128           # 32 K-tiles
NM = 3 * HPC               # 12 m-blocks (4 q, 4 k, 4 v)
QKSCALE = 1.0 / float(np.sqrt(D))

F32 = mybir.dt.float32
BF = mybir.dt.bfloat16
ACT_ID = None  # set in _build
ACT_COPY = None
ACT_EXP = None

# token segments (start, end, expert); segment si pairs with attn chunk si-1
SEGS = [(0, 576, 0), (576, 1088, 1), (1088, 1600, 1), (1600, 2048, 1)]
# v blocks (m 8..11) first so v tiles transpose early; q/k follow
M_ORDER = [8, 9, 10, 11, 0, 1, 2, 3, 4, 5, 6, 7]

_CACHE = {}


def _build():
    global ACT_ID, ACT_COPY, ACT_EXP
    ACT_ID = mybir.ActivationFunctionType.Identity
    ACT_COPY = mybir.ActivationFunctionType.Copy
    ACT_EXP = mybir.ActivationFunctionType.Exp

    nc = bacc.Bacc("TRN2", target_bir_lowering=False, debug=False,
                   num_devices=NCORES)
    dti = nc.dram_tensor
    hs_d = dti("hs", [128, NKT * S], BF, kind="ExternalInput").ap()
    w_d = dti("w", [128, 2 * NM * NKT * 128], BF, kind="ExternalInput").ap()
    wd_d = dti("wd", [128, 2 * 32 * HPC * 128], BF, kind="ExternalInput").ap()
    cos_d = dti("cos", [128, S], BF, kind="ExternalInput").ap()
    sin_d = dti("sin", [128, S], BF, kind="ExternalInput").ap()
    mask_d = dti("mask", [128, 128], BF, kind="ExternalInput").ap()
    rm_d = dti("rm", [D, D], BF, kind="ExternalInput").ap()
    ones_d = dti("ones", [128, 1], BF, kind="ExternalInput").ap()
    onesr_d = dti("onesr", [1, 128], BF, kind="ExternalInput").ap()
    bias_d = dti("bias", [128, NM], F32, kind="ExternalInput").ap()
    out_d = dti("outT", [HID, S], BF, kind="ExternalOutput").ap()

    hs3 = hs_d.tensor.ap().rearrange("p (k t) -> p k t", k=NKT)

    with tile.TileContext(nc) as tc:
        with tc.tile_pool(name="glob", bufs=1) as glob:
            cos_t = glob.tile([128, S], BF)
            sin_t = glob.tile([128, S], BF)
            mask_t = glob.tile([128, 128], BF)
            rm_t = glob.tile([D, D], BF)
            ones_t = glob.tile([128, 1], BF)
            onesr_t = glob.tile([1, 128], BF)
            bias_t = glob.tile([128, NM], F32)
            hs_sliv = glob.tile([128, NKT, 64], BF)

            qT = [glob.tile([128, S], BF, name=f"qT{h}") for h in range(HPC)]
            kT = [glob.tile([128, S], BF, name=f"kT{h}") for h in range(HPC)]
            v_sb = [glob.tile([128, 16, 128], BF, name=f"v{h}")
                    for h in range(HPC)]
            vTb = {b: [glob.tile([128, 128], BF, name=f"vb{b}{h}")
                       for h in range(HPC)] for b in (1, 2, 3)}
            attnT = [glob.tile([128, S], BF, name=f"attnT{h}")
                     for h in range(HPC)]

            # dense weights + attn pools outlive the QKV-only pools
            wd_ctx = tc.tile_pool(name="wdp", bufs=6)
            wd_pool = wd_ctx.__enter__()
            atn_ctx = tc.tile_pool(name="atn", bufs=1)
            atn_pool = atn_ctx.__enter__()
            ps_ctx = tc.tile_pool(name="ps512", bufs=4, space="PSUM")
            ps512 = ps_ctx.__enter__()
            pt_ctx = tc.tile_pool(name="ptp", bufs=2, space="PSUM")
            pt_pool = pt_ctx.__enter__()
            ap_ctx = tc.tile_pool(name="app", bufs=2, space="PSUM")
            ap_pool = ap_ctx.__enter__()

            # ---------------- attention chunk machinery ----------------
            # unit = one keytile of one head: score MM -> exp -> (mask) ->
            # acc -> PV MM. pend keeps a 3-deep score->PV pipeline.
            class AttnChunk:
                def __init__(self, c):
                    self.c = c
                    self.q0 = 512 * c
                    self.units = []
                    for h in range(HPC):
                        for j in range(4):        # diagonal keytiles
                            self.units.append(
                                (h, 4 * c + j, 128 * j, True, j == 0,
                                 c == 0 and j == 3))
                        for jt in range(4 * c):   # off-diagonal keytiles
                            self.units.append(
                                (h, jt, 0, False, False, jt == 4 * c - 1))
                    self.pend = []
                    self.acc = {}
                    self.ap = {}
                    self.idx = 0

                def emit_unit(self, u):
                    (h, ktj, off, diag, first, last) = u
                    w = 512 - off
                    sc = ps512.tile([128, 512], F32, tag="u", name="sc")
                    nc.tensor.matmul(sc[:, off:512],
                                     kT[h][:, 128 * ktj:128 * (ktj + 1)],
                                     qT[h][:, self.q0 + off:self.q0 + 512],
                                     start=True, stop=True)
                    pb = atn_pool.tile([128, 512], BF, tag="pb", bufs=4,
                                       name="pb")
                    nc.scalar.activation(out=pb[:, off:512],
                                         in_=sc[:, off:512],
                                         func=ACT_EXP, scale=QKSCALE)
                    if diag:
                        nc.gpsimd.tensor_mul(pb[:, off:off + 128],
                                             pb[:, off:off + 128], mask_t[:])
                    if first:
                        acc = atn_pool.tile([128, 512], BF, tag="acc",
                                            bufs=2, name="acc")
                        self.acc[h] = acc
                        nc.vector.tensor_copy(acc[:], pb[:])
                    else:
                        acc = self.acc[h]
                        nc.vector.tensor_add(acc[:, off:], acc[:, off:],
                                             pb[:, off:])
                    self.pend.append((h, ktj, off, first, last, pb))
                    if len(self.pend) > 2:
                        self.flush_one()

                def flush_one(self):
                    (h, ktj, off, first, last, pb) = self.pend.pop(0)
                    if first:
                        self.ap[h] = ap_pool.tile([128, 512], F32, tag="ap",
                                                  name="ap")
                    nc.tensor.matmul(self.ap[h][:, off:512],
                                     v_sb[h][:, ktj, :], pb[:, off:512],
                                     start=first, stop=last)
                    if last:
                        self.finish(h)

                def finish(self, h):
                    sp = ps512.tile([128, 512], F32, tag="u", name="sp")
                    nc.tensor.matmul(sp[0:1, :], ones_t[:], self.acc[h][:],
                                     start=True, stop=True)
                    rcf = atn_pool.tile([1, 512], F32, tag="rcf", bufs=2,
                                        name="rcf")
                    nc.vector.reciprocal_approx_fast(out=rcf[:],
                                                     in_=sp[0:1, :])
                    rcb = atn_pool.tile([1, 512], BF, tag="rcb", bufs=2,
                                        name="rcb")
                    nc.vector.tensor_copy(rcb[:], rcf[:])
                    # broadcast rcb across partitions: K=1 outer product
                    rbp = ps512.tile([128, 512], F32, tag="u", name="rbp")
                    nc.tensor.matmul(rbp[:], onesr_t[0:1, :], rcb[0:1, :],
                                     start=True, stop=True)
                    rb = atn_pool.tile([128, 512], BF, tag="rb", bufs=2,
                                       name="rb")
                    nc.vector.tensor_copy(rb[:], rbp[:])
                    nc.vector.tensor_mul(
                        attnT[h][:, self.q0:self.q0 + 512],
                        self.ap[h][:], rb[:])

                def pump_some(self, n):
                    used = 0
                    while used < n:
                        if self.idx < len(self.units):
                            self.emit_unit(self.units[self.idx])
                            self.idx += 1
                            used += 1
                        elif self.pend:
                            self.flush_one()
                            used += 1
                        else:
                            break
                    return used

                def done(self):
                    return self.idx >= len(self.units) and not self.pend

            # ---------------- QKV emission ----------------
            def load_w(e, m):
                wt = tc_w_pool.tile([128, NKT, 128], BF, tag="w", name="wt")
                base = (e * NM + m) * NKT * 128
                # two halves so the m-block's first matmuls start early
                nc.sync.dma_start(out=wt[:, 0:16, :],
                                  in_=w_d[:, base:base + 16 * 128])
                nc.sync.dma_start(out=wt[:, 16:32, :],
                                  in_=w_d[:, base + 16 * 128:base + NKT * 128])
                return wt

            def rope(m, qk_sb, c0, w):
                rot = ps512.tile([128, 512], F32, tag="u", name="rot")
                nc.tensor.matmul(rot[:, :w], rm_t[:], qk_sb[:, :w],
                                 start=True, stop=True)
                prod = tc_ev_pool.tile([128, 512], BF, tag="prod", bufs=2,
                                       name="prod")
                nc.vector.tensor_mul(prod[:, :w], qk_sb[:, :w],
                                     cos_t[:, c0:c0 + w])
                rp = tc_ev_pool.tile([128, 512], BF, tag="rp", bufs=2,
                                     name="rp")
                nc.vector.tensor_mul(rp[:, :w], rot[:, :w],
                                     sin_t[:, c0:c0 + w])
                tgt = qT[m] if m < HPC else kT[m - HPC]
                nc.vector.tensor_add(tgt[:, c0:c0 + w], prod[:, :w],
                                     rp[:, :w])

            def evac_qk(m, e, pt, w, c0):
                qk_sb = tc_ev_pool.tile([128, 512], BF, tag="qksb", bufs=2,
                                        name="qk_sb")
                if e == 0:
                    nc.scalar.activation(out=qk_sb[:, :w], in_=pt[:, :w],
                                         func=ACT_ID,
                                         bias=bias_t[:, m:m + 1], scale=1.0)
                else:
                    nc.scalar.activation(out=qk_sb[:, :w], in_=pt[:, :w],
                                         func=ACT_COPY, scale=1.0)
                rope(m, qk_sb, c0, w)

            def evac_v(m, e, pt, si, w):
                # seg si covers tokens [s0, s0+w); for si>0, s0 % 128 == 64.
                mv = m - 2 * HPC
                o1 = 64 if si else 0
                stg = tc_ev_pool.tile([128, 576], BF, tag="vstg", bufs=3,
                                      name="stg")
                if e == 0:
                    nc.scalar.activation(out=stg[:, o1:o1 + w],
                                         in_=pt[:, :w], func=ACT_ID,
                                         bias=bias_t[:, m:m + 1], scale=1.0)
                else:
                    nc.scalar.activation(out=stg[:, o1:o1 + w],
                                         in_=pt[:, :w], func=ACT_COPY,
                                         scale=1.0)
                # transposes are DEFERRED one m-block (returned as a closure)
                # so their input-ready waits never head-of-line block the
                # sync DMA FIFO in front of weight/hs streams.
                # split the 4 transposes across both HWDGE rings so neither
                # the weight stream (sync) nor the evac/exp queue (scalar)
                # eats the full burst
                if si == 0:
                    def tp():
                        for jt in range(4):
                            eng = nc.sync if jt % 2 == 0 else nc.scalar
                            eng.dma_start_transpose(
                                out=v_sb[mv][:, jt, :],
                                in_=stg[:, 128 * jt:128 * (jt + 1)])
                else:
                    b = si
                    nc.vector.tensor_copy(vTb[b][mv][:, 64:128],
                                          stg[:, 64:128])
                    if b + 1 <= 3 and w == 512:
                        nc.vector.tensor_copy(vTb[b + 1][mv][:, 0:64],
                                              stg[:, 576 - 64:576])

                    def tp():
                        nc.sync.dma_start_transpose(
                            out=v_sb[mv][:, 4 * b, :], in_=vTb[b][mv][:])
                        for i in range(3):
                            eng = nc.scalar if i % 2 == 0 else nc.sync
                            eng.dma_start_transpose(
                                out=v_sb[mv][:, 4 * b + 1 + i, :],
                                in_=stg[:, 128 * (i + 1):128 * (i + 2)])
                return tp

            with tc.tile_pool(name="hsp", bufs=2) as hs_pool, \
                 tc.tile_pool(name="wp", bufs=2) as tc_w_pool, \
                 tc.tile_pool(name="evp", bufs=2) as tc_ev_pool:

                # seg0 streams first so the PE starts ASAP; constants load
                # behind them.
                # quarter-granularity first weight load: the very first
                # matmul only needs kt 0..7 resident
                wt0 = tc_w_pool.tile([128, NKT, 128], BF, tag="w", name="wt")
                b0 = (0 * NM + 8) * NKT * 128
                for q in range(4):
                    nc.sync.dma_start(
                        out=wt0[:, 8 * q:8 * (q + 1), :],
                        in_=w_d[:, b0 + q * 1024:b0 + (q + 1) * 1024])
                    if q == 0:
                        hst0 = hs_pool.tile([128, NKT, 512], BF, tag="hs",
                                            name="hst")
                        nc.sync.dma_start(out=hst0[:, 0:8, :],
                                          in_=hs3[:, 0:8, 0:512])
                        nc.sync.dma_start(out=hs_sliv[:, 0:8, :],
                                          in_=hs3[:, 0:8, 512:576])
                wpre = {(0, 8): wt0}
                for kg in range(8, NKT, 8):
                    nc.sync.dma_start(out=hst0[:, kg:kg + 8, :],
                                      in_=hs3[:, kg:kg + 8, 0:512])
                nc.sync.dma_start(out=hs_sliv[:, 8:, :],
                                  in_=hs3[:, 8:, 512:576])
                nc.sync.dma_start(out=bias_t[:], in_=bias_d[:])
                nc.sync.dma_start(out=rm_t[:], in_=rm_d[:])
                wpre[(0, 9)] = load_w(0, 9)
                nc.sync.dma_start(out=cos_t[:, :576], in_=cos_d[:, :576])
                nc.sync.dma_start(out=sin_t[:, :576], in_=sin_d[:, :576])
                nc.sync.dma_start(out=ones_t[:], in_=ones_d[:])
                nc.sync.dma_start(out=onesr_t[:], in_=onesr_d[:])
                nc.sync.dma_start(out=mask_t[:], in_=mask_d[:])
                nc.sync.dma_start(out=cos_t[:, 576:], in_=cos_d[:, 576:])
                nc.sync.dma_start(out=sin_t[:, 576:], in_=sin_d[:, 576:])
                # warm the exp table set early (one-element activation)
                exp_warm = tc_ev_pool.tile([1, 1], F32, tag="ew", bufs=1,
                                           name="exp_warm")
                nc.scalar.activation(out=exp_warm[:], in_=bias_t[0:1, 0:1],
                                     func=ACT_EXP, scale=1.0)

                hs_cur = hst0
                active = []          # attention chunks with remaining work
                pend_tp = []         # deferred v-transpose closures
                wd_loaded = []

                def pump_slot(n):
                    while n > 0 and active:
                        used = active[0].pump_some(n)
                        if active[0].done():
                            active.pop(0)
                        if used == 0 and not active:
                            break
                        n -= used

                for si, (s0, s1, e) in enumerate(SEGS):
                    sw = s1 - s0 if si else 512
                    hst = hs_cur
                    if si + 1 < len(SEGS):
                        n0, n1, _ = SEGS[si + 1]
                        hs_cur = hs_pool.tile([128, NKT, 512], BF, tag="hs",
                                              name="hst")
                    if si >= 1:
                        active.append(AttnChunk(si - 1))

                    for mi, m in enumerate(M_ORDER):
                        # flush deferred transposes from the previous block
                        for tp in pend_tp:
                            tp()
                        pend_tp = []
                        # spread next segment's hs prefetch into the m-loop
                        if si + 1 < len(SEGS) and mi in (1, 3, 5, 7):
                            kg = 8 * (mi // 2)
                            nc.sync.dma_start(
                                out=hs_cur[:, kg:kg + 8, :n1 - n0],
                                in_=hs3[:, kg:kg + 8, n0:n1])
                        wt = wpre.pop((e, m), None)
                        if wt is None:
                            wt = load_w(e, m)
                        # prefetch 1 m-block ahead (same or next segment)
                        pf = mi + 1
                        if pf < len(M_ORDER):
                            key = (e, M_ORDER[pf])
                            if key not in wpre:
                                wpre[key] = load_w(*key)
                        elif si + 1 < len(SEGS):
                            key = (SEGS[si + 1][2], M_ORDER[pf - len(M_ORDER)])
                            if key not in wpre:
                                wpre[key] = load_w(*key)

                        ptA = pt_pool.tile([128, 512], F32, tag="pt",
                                           name="ptA")
                        ptB = None
                        if si == 0:
                            # the ap pool is idle during seg0 (no attention)
                            ptB = ap_pool.tile([128, 512], F32, tag="ap",
                                               name="ptB")
                        for half in range(2):
                            k0 = 16 * half
                            for kt in range(k0, k0 + 16):
                                nc.tensor.matmul(ptA[:, :sw], wt[:, kt, :],
                                                 hst[:, kt, :sw],
                                                 start=(kt == 0),
                                                 stop=(kt == NKT - 1))
                                if ptB is not None:
                                    nc.tensor.matmul(
                                        ptB[:, :64], wt[:, kt, :],
                                        hs_sliv[:, kt, :],
                                        start=(kt == 0),
                                        stop=(kt == NKT - 1))
                            pump_slot(3)
                        if m < 2 * HPC:
                            evac_qk(m, e, ptA, sw, s0)
                            if ptB is not None:
                                qkb = tc_ev_pool.tile([128, 64], BF,
                                                      tag="qkb", bufs=2,
                                                      name="qkb")
                                nc.scalar.activation(
                                    out=qkb[:], in_=ptB[:, :64],
                                    func=ACT_ID,
                                    bias=bias_t[:, m:m + 1], scale=1.0)
                                rope(m, qkb, 512, 64)
                        else:
                            pend_tp.append(evac_v(m, e, ptA, si, sw))
                            if ptB is not None:
                                mv = m - 2 * HPC
                                nc.scalar.activation(
                                    out=vTb[1][mv][:, 0:64],
                                    in_=ptB[:, :64], func=ACT_ID,
                                    bias=bias_t[:, m:m + 1], scale=1.0)

                    for tp in pend_tp:
                        tp()
                    pend_tp = []

                    if si == 3:
                        # finish any remaining chunk-2 work before dense
                        while active:
                            active[0].pump_some(10**6)
                            active.pop(0)
                        # prefetch first dense weight groups (4-o tiles)
                        for og in range(2):
                            for ee in range(2):
                                wdt = wd_pool.tile([128, 4, HPC, 128], BF,
                                                   tag="wd", name="wdt")
                                base = (ee * 32 + 4 * og) * HPC * 128
                                nc.sync.dma_start(
                                    out=wdt[:],
                                    in_=wd_d[:, base:base + 4 * HPC * 128])
                                wd_loaded.append(((ee, og), wdt))

            # ---------------- dense phase ----------------
            # pass A: tokens 0..1536 (interleaved with attn chunk 3),
            # pass B: tokens 1536..2048 (needs chunk 3's attnT).
            DCH_A = [[(0, 512), (512, 576)], [(576, 1088), (1088, 1536)]]
            last_chunk = AttnChunk(3)
            wd_tiles = dict(wd_loaded)

            def load_wd(ee, og):
                t = wd_pool.tile([128, 4, HPC, 128], BF, tag="wd",
                                 name="wdt")
                base = (ee * 32 + 4 * og) * HPC * 128
                nc.sync.dma_start(
                    out=t[:], in_=wd_d[:, base:base + 4 * HPC * 128])
                return t

            def get_wd(ee, og):
                t = wd_tiles.get((ee, og))
                if t is None:
                    t = load_wd(ee, og)
                    wd_tiles[(ee, og)] = t
                return t

            with tc.tile_pool(name="oep", bufs=3) as oe_pool:
                nsub = 32
                usched = [len(last_chunk.units) // nsub +
                          (1 if i < len(last_chunk.units) % nsub else 0)
                          for i in range(nsub)]
                sub_i = 0
                ei = 0
                for og in range(8):
                    for ee in range(2):
                        get_wd(ee, og)
                    if og + 2 < 8:
                        for ee in range(2):
                            get_wd(ee, og + 2)
                    for oi in range(HPC):
                        o = 4 * og + oi
                        oe = oe_pool.tile([128, 1536], BF, tag="oe",
                                          name="oe")
                        for ee in range(2):
                            wdt = get_wd(ee, og)
                            pos = [pt_pool.tile([128, 512], F32, tag="pt",
                                                name="po")
                                   for _ in DCH_A[ee]]
                            for hh in range(HPC):
                                for ci, (t0, t1) in enumerate(DCH_A[ee]):
                                    nc.tensor.matmul(
                                        pos[ci][:, :t1 - t0],
                                        wdt[:, oi, hh, :],
                                        attnT[hh][:, t0:t1],
                                        start=(hh == 0),
                                        stop=(hh == HPC - 1))
                            for ci, (t0, t1) in enumerate(DCH_A[ee]):
                                if ei % 2 == 0:
                                    nc.scalar.activation(
                                        out=oe[:, t0:t1],
                                        in_=pos[ci][:, :t1 - t0],
                                        func=ACT_COPY, scale=1.0)
                                else:
                                    nc.vector.tensor_copy(
                                        oe[:, t0:t1], pos[ci][:, :t1 - t0])
                                ei += 1
                        nc.scalar.dma_start(
                            out=out_d[128 * o:128 * (o + 1), 0:1536],
                            in_=oe[:])
                        last_chunk.pump_some(usched[sub_i])
                        sub_i += 1
                last_chunk.pump_some(10**6)

                # pass B — expert-1 tiles were recycled by the ring; reload.
                # accumulators alternate between the pt and (now idle) ap
                # pools for 4-deep psum pipelining.
                wdB = {0: load_wd(1, 0), 1: load_wd(1, 1)}
                for og in range(8):
                    wdt = wdB.pop(og)
                    if og + 2 < 8:
                        wdB[og + 2] = load_wd(1, og + 2)
                    for oi in range(HPC):
                        o = 4 * og + oi
                        if oi % 2 == 0:
                            po = pt_pool.tile([128, 512], F32, tag="pt",
                                              name="poB")
                        else:
                            po = ap_pool.tile([128, 512], F32, tag="ap",
                                              name="poB")
                        for hh in range(HPC):
                            nc.tensor.matmul(po[:], wdt[:, oi, hh, :],
                                             attnT[hh][:, 1536:2048],
                                             start=(hh == 0),
                                             stop=(hh == HPC - 1))
                        oe2 = oe_pool.tile([128, 512], BF, tag="oe2",
                                           name="oe2")
                        if ei % 2 == 0:
                            nc.scalar.activation(out=oe2[:], in_=po[:],
                                                 func=ACT_COPY, scale=1.0)
                        else:
                            nc.vector.tensor_copy(oe2[:], po[:])
                        ei += 1
                        nc.sync.dma_start(
                            out=out_d[128 * o:128 * (o + 1), 1536:2048],
                            in_=oe2[:])

            ap_ctx.__exit__(None, None, None)
            pt_ctx.__exit__(None, None, None)
            ps_ctx.__exit__(None, None, None)
            atn_ctx.__exit__(None, None, None)
            wd_ctx.__exit__(None, None, None)
    nc.compile()
    return nc


def _prep_inputs(inputs):
    import ml_dtypes
    bf = ml_dtypes.bfloat16

    hs = np.asarray(inputs["hidden_states"], np.float32)
    cos = np.asarray(inputs["cos"], np.float32)
    sin = np.asarray(inputs["sin"], np.float32)
    vi = np.asarray(inputs["vision_indices"]).ravel()
    li = np.asarray(inputs["language_indices"]).ravel()
    assert vi.size == NV and np.array_equal(vi, np.arange(NV)) and \
        np.array_equal(li, np.arange(NV, S)), "unsupported index layout"

    # hs tiled [128, (kt, t)]
    hs_t = np.ascontiguousarray(
        hs.T.reshape(NKT, 128, S).transpose(1, 0, 2).reshape(128, NKT * S)
    ).astype(bf)

    cos_t = np.ascontiguousarray(cos.T).astype(bf)
    sin_t = np.ascontiguousarray(sin.T).astype(bf)

    rm = np.zeros((D, D), np.float32)
    for d in range(64):
        rm[d + 64, d] = -1.0
        rm[d, d + 64] = 1.0
    rm = rm.astype(bf)

    # mask[p, q'] = 1 iff p <= q' within a 128 diagonal block
    p = np.arange(128)[:, None]
    q = np.arange(128)[None, :]
    mask = (p <= q).astype(np.float32).astype(bf)

    ones = np.ones((128, 1), np.float32).astype(bf)

    b = np.asarray(inputs["vision_qkv_b"], np.float32)
    Wqkv = np.stack([np.asarray(inputs["vision_qkv_w"], np.float32),
                     np.asarray(inputs["lang_qkv_w"], np.float32)])
    Wd = np.stack([np.asarray(inputs["vision_dense_w"], np.float32),
                   np.asarray(inputs["lang_dense_w"], np.float32)])

    in_maps = []
    for rr in range(NCORES):
        q0 = 512 * rr
        cols = np.r_[q0:q0 + 512, HID + q0:HID + q0 + 512,
                     2 * HID + q0:2 * HID + q0 + 512]
        # w tiled [128, (e, m, kt, c)]
        wc = Wqkv[:, :, cols]                                # [2, 4096, 1536]
        w_t = np.ascontiguousarray(
            wc.reshape(2, NKT, 128, NM, 128)
              .transpose(2, 0, 3, 1, 4)
              .reshape(128, 2 * NM * NKT * 128)).astype(bf)
        # wd tiled [128, (e, o, hh, c)]
        wdc = Wd[:, q0:q0 + 512, :]                          # [2, 512, 4096]
        wd_t = np.ascontiguousarray(
            wdc.reshape(2, HPC, 128, 32, 128)
               .transpose(2, 0, 3, 1, 4)
               .reshape(128, 2 * 32 * HPC * 128)).astype(bf)
        bias_t = np.ascontiguousarray(
            b[cols].reshape(NM, 128).T).astype(np.float32)
        in_maps.append({
            "hs": hs_t, "w": w_t, "wd": wd_t,
            "cos": cos_t, "sin": sin_t, "mask": mask, "rm": rm,
            "ones": ones, "onesr": np.ones((1, 128), np.float32).astype(bf),
            "bias": bias_t,
        })
    return in_maps


def kernel(**inputs):
    if "nc" not in _CACHE:
        _CACHE["nc"] = _build()
    nc = _CACHE["nc"]
    in_maps = _prep_inputs(inputs)
    res = run_bass_kernel_spmd(nc, in_maps, list(range(NCORES)),
                               **_CACHE.get("run_kwargs", {}))
    _CACHE["last_results"] = res
    out = np.zeros((HID, S), np.float32)
    for r in range(NCORES):
        out += res.results[r]["outT"].astype(np.float32)
    return np.ascontiguousarray(out.T)


# revision 33
# speedup vs baseline: 1.1001x; 1.1001x over previous
"""Trainium2 Bass kernel for modality-routed (CogVLM-style) attention, v3.

Contract: kernel(**inputs) takes FULL unsharded numpy inputs (as produced by
the reference's setup_inputs) and returns the FULL [2048, 4096] fp32 output.

Sharding: tensor-parallel over heads. Core r owns heads 4r..4r+3:
  - qkv weights column-sharded; q/k computed in transposed [dim, token]
    orientation via stationary weight tiles; v computed the same way then
    flipped to natural [token, dim] per 128-token tile with DMA transposes.
  - dense weights row-sharded; each core emits a partial output in transposed
    [4096, 2048] orientation, summed + transposed on the host.

v3 structural changes vs v2 (driven by the 880us trace):
  - attention is emitted at keytile granularity and INTERLEAVED between the
    next segment's QKV half-m-blocks, so exp (ACT) latency hides behind QKV
    matmuls instead of stalling the PE (attn windows were 45% PE-idle).
  - diagonal keytiles use restricted query widths (512/384/256/128) plus a
    single 128x128 triangular mask, cutting masked-out score/PV/exp work.
  - softmax normalization: ones-matmul partition sum -> reciprocal_approx_fast
    (vector.reciprocal was 3.3us per call) -> gpsimd partition_broadcast
    (replaces a DRAM bounce round-trip per head).
  - seg0's 64-token vision sliver shares LDWEIGHTS with the 512-token chunk
    (kt-major loop, two PSUM accumulators) instead of paying a LDW-bound pass.
  - DMAs are batched (1 per weight tile / segment / output row-block) and
    split across both HWDGE rings (plain on sync, transposes on scalar);
    the sync ring carried 496 serialized DMAs before.
  - dense runs chunk-major in two passes: tokens 0..1536 interleave with the
    last attention chunk, tokens 1536..2048 follow it; dense weights stream
    in 4-o-group tiles and output rows batch into 2 DMAs per row-block.
"""

import sys

import numpy as np

if "/opt/trn_rl_repo" not in sys.path:
    sys.path.insert(0, "/opt/trn_rl_repo")

import concourse.bass as bass  # noqa: E402,F401
import concourse.tile as tile  # noqa: E402
from concourse import bacc, mybir  # noqa: E402
from concourse.bass_utils import run_bass_kernel_spmd  # noqa: E402

S = 2048
HID = 4096
H = 32
D = 128
NCORES = 8
HPC = H // NCORES          # heads per core = 4
NV = 576                   # vision tokens occupy rows [0, NV)
NKT = HID // 128           # 32 K-tiles
NM = 3 * HPC               # 12 m-blocks (4 q, 4 k, 4 v)
QKSCALE = 1.0 / float(np.sqrt(D))

F32 = mybir.dt.float32
BF = mybir.dt.bfloat16
ACT_ID = None  # set in _build
ACT_COPY = None
ACT_EXP = None

# token segments (start, end, expert); segment si pairs with attn chunk si-1
SEGS = [(0, 576, 0), (576, 1088, 1), (1088, 1600, 1), (1600, 2048, 1)]
# v blocks (m 8..11) first so v tiles transpose early; q/k follow
M_ORDER = [8, 9, 10, 11, 0, 1, 2, 3, 4, 5, 6, 7]

_CACHE = {}


def _build():
    global ACT_ID, ACT_COPY, ACT_EXP
    ACT_ID = mybir.ActivationFunctionType.Identity
    ACT_COPY = mybir.ActivationFunctionType.Copy
    ACT_EXP = mybir.ActivationFunctionType.Exp

    nc = bacc.Bacc("TRN2", target_bir_lowering=False, debug=False,
                   num_devices=NCORES)
    dti = nc.dram_tensor
    hs_d = dti("hs", [128, NKT * S], BF, kind="ExternalInput").ap()
    w_d = dti("w", [128, 2 * NM * NKT * 128], BF, kind="ExternalInput").ap()
    wd_d = dti("wd", [128, 2 * 32 * HPC * 128], BF, kind="ExternalInput").ap()
    cos_d = dti("cos", [128, S], BF, kind="ExternalInput").ap()
    sin_d = dti("sin", [128, S], BF, kind="ExternalInput").ap()
    mask_d = dti("mask", [128, 128], BF, kind="ExternalInput").ap()
    rm_d = dti("rm", [D, D], BF, kind="ExternalInput").ap()
    ones_d = dti("ones", [128, 1], BF, kind="ExternalInput").ap()
    onesr_d = dti("onesr", [1, 128], BF, kind="ExternalInput").ap()
    bias_d = dti("bias", [128, NM], F32, kind="ExternalInput").ap()
    out_d = dti("outT", [HID, S], BF, kind="ExternalOutput").ap()

    hs3 = hs_d.tensor.ap().rearrange("p (k t) -> p k t", k=NKT)

    with tile.TileContext(nc) as tc:
        with tc.tile_pool(name="glob", bufs=1) as glob:
            cos_t = glob.tile([128, S], BF)
            sin_t = glob.tile([128, S], BF)
            mask_t = glob.tile([128, 128], BF)
            rm_t = glob.tile([D, D], BF)
            ones_t = glob.tile([128, 1], BF)
            onesr_t = glob.tile([1, 128], BF)
            bias_t = glob.tile([128, NM], F32)
            hs_sliv = glob.tile([128, NKT, 64], BF)

            qT = [glob.tile([128, S], BF, name=f"qT{h}") for h in range(HPC)]
            kT = [glob.tile([128, S], BF, name=f"kT{h}") for h in range(HPC)]
            v_sb = [glob.tile([128, 16, 128], BF, name=f"v{h}")
                    for h in range(HPC)]
            vTb = {b: [glob.tile([128, 128], BF, name=f"vb{b}{h}")
                       for h in range(HPC)] for b in (1, 2, 3)}
            attnT = [glob.tile([128, S], BF, name=f"attnT{h}")
                     for h in range(HPC)]

            # dense weights + attn pools outlive the QKV-only pools
            wd_ctx = tc.tile_pool(name="wdp", bufs=6)
            wd_pool = wd_ctx.__enter__()
            atn_ctx = tc.tile_pool(name="atn", bufs=1)
            atn_pool = atn_ctx.__enter__()
            ps_ctx = tc.tile_pool(name="ps512", bufs=4, space="PSUM")
            ps512 = ps_ctx.__enter__()
            pt_ctx = tc.tile_pool(name="ptp", bufs=2, space="PSUM")
            pt_pool = pt_ctx.__enter__()
            ap_ctx = tc.tile_pool(name="app", bufs=2, space="PSUM")
            ap_pool = ap_ctx.__enter__()

            # ---------------- attention chunk machinery ----------------
            # unit = one keytile of one head: score MM -> exp -> (mask) ->
            # acc -> PV MM. pend keeps a 3-deep score->PV pipeline.
            class AttnChunk:
                def __init__(self, c):
                    self.c = c
                    self.q0 = 512 * c
                    self.units = []
                    for h in range(HPC):
                        for j in range(4):        # diagonal keytiles
                            self.units.append(
                                (h, 4 * c + j, 128 * j, True, j == 0,
                                 c == 0 and j == 3))
                        for jt in range(4 * c):   # off-diagonal keytiles
                            self.units.append(
                                (h, jt, 0, False, False, jt == 4 * c - 1))
                    self.pend = []
                    self.acc = {}
                    self.ap = {}
                    self.idx = 0

                def emit_unit(self, u):
                    (h, ktj, off, diag, first, last) = u
                    w = 512 - off
                    sc = ps512.tile([128, 512], F32, tag="u", name="sc")
                    nc.tensor.matmul(sc[:, off:512],
                                     kT[h][:, 128 * ktj:128 * (ktj + 1)],
                                     qT[h][:, self.q0 + off:self.q0 + 512],
                                     start=True, stop=True)
                    pb = atn_pool.tile([128, 512], BF, tag="pb", bufs=4,
                                       name="pb")
                    nc.scalar.activation(out=pb[:, off:512],
                                         in_=sc[:, off:512],
                                         func=ACT_EXP, scale=QKSCALE)
                    if diag:
                        nc.gpsimd.tensor_mul(pb[:, off:off + 128],
                                             pb[:, off:off + 128], mask_t[:])
                    if first:
                        acc = atn_pool.tile([128, 512], BF, tag="acc",
                                            bufs=2, name="acc")
                        self.acc[h] = acc
                        nc.vector.tensor_copy(acc[:], pb[:])
                    else:
                        acc = self.acc[h]
                        nc.vector.tensor_add(acc[:, off:], acc[:, off:],
                                             pb[:, off:])
                    self.pend.append((h, ktj, off, first, last, pb))
                    if len(self.pend) > 2:
                        self.flush_one()

                def flush_one(self):
                    (h, ktj, off, first, last, pb) = self.pend.pop(0)
                    if first:
                        self.ap[h] = ap_pool.tile([128, 512], F32, tag="ap",
                                                  name="ap")
                    nc.tensor.matmul(self.ap[h][:, off:512],
                                     v_sb[h][:, ktj, :], pb[:, off:512],
                                     start=first, stop=last)
                    if last:
                        self.finish(h)

                def finish(self, h):
                    sp = ps512.tile([128, 512], F32, tag="u", name="sp")
                    nc.tensor.matmul(sp[0:1, :], ones_t[:], self.acc[h][:],
                                     start=True, stop=True)
                    rcf = atn_pool.tile([1, 512], F32, tag="rcf", bufs=2,
                                        name="rcf")
                    nc.vector.reciprocal_approx_fast(out=rcf[:],
                                                     in_=sp[0:1, :])
                    rcb = atn_pool.tile([1, 512], BF, tag="rcb", bufs=2,
                                        name="rcb")
                    nc.vector.tensor_copy(rcb[:], rcf[:])
                    # broadcast rcb across partitions: K=1 outer product
                    rbp = ps512.tile([128, 512], F32, tag="u", name="rbp")
                    nc.tensor.matmul(rbp[:], onesr_t[0:1, :], rcb[0:1, :],
                                     start=True, stop=True)
                    rb = atn_pool.tile([128, 512], BF, tag="rb", bufs=2,
                                       name="rb")
                    nc.vector.tensor_copy(rb[:], rbp[:])
                    nc.vector.tensor_mul(
                        attnT[h][:, self.q0:self.q0 + 512],
                        self.ap[h][:], rb[:])

                def pump_some(self, n):
                    used = 0
                    while used < n:
                        if self.idx < len(self.units):
                            self.emit_unit(self.units[self.idx])
                            self.idx += 1
                            used += 1
                        elif self.pend:
                            self.flush_one()
                            used += 1
                        else:
                            break
                    return used

                def done(self):
                    return self.idx >= len(self.units) and not self.pend

            # ---------------- QKV emission ----------------
            def load_w(e, m):
                wt = tc_w_pool.tile([128, NKT, 128], BF, tag="w", name="wt")
                base = (e * NM + m) * NKT * 128
                # two halves so the m-block's first matmuls start early
                nc.sync.dma_start(out=wt[:, 0:16, :],
                                  in_=w_d[:, base:base + 16 * 128])
                nc.sync.dma_start(out=wt[:, 16:32, :],
                                  in_=w_d[:, base + 16 * 128:base + NKT * 128])
                return wt

            def rope(m, qk_sb, c0, w):
                rot = ps512.tile([128, 512], F32, tag="u", name="rot")
                nc.tensor.matmul(rot[:, :w], rm_t[:], qk_sb[:, :w],
                                 start=True, stop=True)
                prod = tc_ev_pool.tile([128, 512], BF, tag="prod", bufs=2,
                                       name="prod")
                nc.vector.tensor_mul(prod[:, :w], qk_sb[:, :w],
                                     cos_t[:, c0:c0 + w])
                rp = tc_ev_pool.tile([128, 512], BF, tag="rp", bufs=2,
                                     name="rp")
                nc.vector.tensor_mul(rp[:, :w], rot[:, :w],
                                     sin_t[:, c0:c0 + w])
                tgt = qT[m] if m < HPC else kT[m - HPC]
                nc.vector.tensor_add(tgt[:, c0:c0 + w], prod[:, :w],
                                     rp[:, :w])

            def evac_qk(m, e, pt, w, c0):
                qk_sb = tc_ev_pool.tile([128, 512], BF, tag="qksb", bufs=2,
                                        name="qk_sb")
                if e == 0:
                    nc.scalar.activation(out=qk_sb[:, :w], in_=pt[:, :w],
                                         func=ACT_ID,
                                         bias=bias_t[:, m:m + 1], scale=1.0)
                else:
                    nc.scalar.activation(out=qk_sb[:, :w], in_=pt[:, :w],
                                         func=ACT_COPY, scale=1.0)
                rope(m, qk_sb, c0, w)

            def evac_v(m, e, pt, si, w):
                # seg si covers tokens [s0, s0+w); for si>0, s0 % 128 == 64.
                mv = m - 2 * HPC
                o1 = 64 if si else 0
                stg = tc_ev_pool.tile([128, 576], BF, tag="vstg", bufs=3,
                                      name="stg")
                if e == 0:
                    nc.scalar.activation(out=stg[:, o1:o1 + w],
                                         in_=pt[:, :w], func=ACT_ID,
                                         bias=bias_t[:, m:m + 1], scale=1.0)
                else:
                    nc.scalar.activation(out=stg[:, o1:o1 + w],
                                         in_=pt[:, :w], func=ACT_COPY,
                                         scale=1.0)
                # transposes are DEFERRED one m-block (returned as a closure)
                # so their input-ready waits never head-of-line block the
                # sync DMA FIFO in front of weight/hs streams.
                # split the 4 transposes across both HWDGE rings so neither
                # the weight stream (sync) nor the evac/exp queue (scalar)
                # eats the full burst
                if si == 0:
                    def tp():
                        for jt in range(4):
                            nc.sync.dma_start_transpose(
                                out=v_sb[mv][:, jt, :],
                                in_=stg[:, 128 * jt:128 * (jt + 1)])
                else:
                    b = si
                    nc.vector.tensor_copy(vTb[b][mv][:, 64:128],
                                          stg[:, 64:128])
                    if b + 1 <= 3 and w == 512:
                        nc.vector.tensor_copy(vTb[b + 1][mv][:, 0:64],
                                              stg[:, 576 - 64:576])

                    def tp():
                        nc.sync.dma_start_transpose(
                            out=v_sb[mv][:, 4 * b, :], in_=vTb[b][mv][:])
                        for i in range(3):
                            nc.sync.dma_start_transpose(
                                out=v_sb[mv][:, 4 * b + 1 + i, :],
                                in_=stg[:, 128 * (i + 1):128 * (i + 2)])
                return tp

            with tc.tile_pool(name="hsp", bufs=2) as hs_pool, \
                 tc.tile_pool(name="wp", bufs=2) as tc_w_pool, \
                 tc.tile_pool(name="evp", bufs=2) as tc_ev_pool:

                # seg0 streams first so the PE starts ASAP; constants load
                # behind them.
                # quarter-granularity first weight load: the very first
                # matmul only needs kt 0..7 resident
                wt0 = tc_w_pool.tile([128, NKT, 128], BF, tag="w", name="wt")
                b0 = (0 * NM + 8) * NKT * 128
                for q in range(4):
                    nc.sync.dma_start(
                        out=wt0[:, 8 * q:8 * (q + 1), :],
                        in_=w_d[:, b0 + q * 1024:b0 + (q + 1) * 1024])
                    if q == 0:
                        hst0 = hs_pool.tile([128, NKT, 512], BF, tag="hs",
                                            name="hst")
                        nc.sync.dma_start(out=hst0[:, 0:8, :],
                                          in_=hs3[:, 0:8, 0:512])
                        nc.sync.dma_start(out=hs_sliv[:, 0:8, :],
                                          in_=hs3[:, 0:8, 512:576])
                wpre = {(0, 8): wt0}
                for kg in range(8, NKT, 8):
                    nc.sync.dma_start(out=hst0[:, kg:kg + 8, :],
                                      in_=hs3[:, kg:kg + 8, 0:512])
                nc.sync.dma_start(out=hs_sliv[:, 8:, :],
                                  in_=hs3[:, 8:, 512:576])
                nc.sync.dma_start(out=bias_t[:], in_=bias_d[:])
                nc.sync.dma_start(out=rm_t[:], in_=rm_d[:])
                wpre[(0, 9)] = load_w(0, 9)
                nc.sync.dma_start(out=cos_t[:, :576], in_=cos_d[:, :576])
                nc.sync.dma_start(out=sin_t[:, :576], in_=sin_d[:, :576])
                nc.sync.dma_start(out=ones_t[:], in_=ones_d[:])
                nc.sync.dma_start(out=onesr_t[:], in_=onesr_d[:])
                nc.sync.dma_start(out=mask_t[:], in_=mask_d[:])
                nc.sync.dma_start(out=cos_t[:, 576:], in_=cos_d[:, 576:])
                nc.sync.dma_start(out=sin_t[:, 576:], in_=sin_d[:, 576:])
                # warm the exp table set early (one-element activation)
                exp_warm = tc_ev_pool.tile([1, 1], F32, tag="ew", bufs=1,
                                           name="exp_warm")
                nc.scalar.activation(out=exp_warm[:], in_=bias_t[0:1, 0:1],
                                     func=ACT_EXP, scale=1.0)

                hs_cur = hst0
                active = []          # attention chunks with remaining work
                pend_tp = []         # deferred v-transpose closures
                wd_loaded = []

                def pump_slot(n):
                    while n > 0 and active:
                        used = active[0].pump_some(n)
                        if active[0].done():
                            active.pop(0)
                        if used == 0 and not active:
                            break
                        n -= used

                for si, (s0, s1, e) in enumerate(SEGS):
                    sw = s1 - s0 if si else 512
                    hst = hs_cur
                    if si + 1 < len(SEGS):
                        n0, n1, _ = SEGS[si + 1]
                        hs_cur = hs_pool.tile([128, NKT, 512], BF, tag="hs",
                                              name="hst")
                    if si >= 1:
                        active.append(AttnChunk(si - 1))

                    for mi, m in enumerate(M_ORDER):
                        # flush deferred transposes from the previous block
                        for tp in pend_tp:
                            tp()
                        pend_tp = []
                        # spread next segment's hs prefetch into the m-loop
                        if si + 1 < len(SEGS) and mi in (1, 3, 5, 7):
                            kg = 8 * (mi // 2)
                            nc.sync.dma_start(
                                out=hs_cur[:, kg:kg + 8, :n1 - n0],
                                in_=hs3[:, kg:kg + 8, n0:n1])
                        wt = wpre.pop((e, m), None)
                        if wt is None:
                            wt = load_w(e, m)
                        # prefetch 1 m-block ahead (same or next segment)
                        pf = mi + 1
                        if pf < len(M_ORDER):
                            key = (e, M_ORDER[pf])
                            if key not in wpre:
                                wpre[key] = load_w(*key)
                        elif si + 1 < len(SEGS):
                            key = (SEGS[si + 1][2], M_ORDER[pf - len(M_ORDER)])
                            if key not in wpre:
                                wpre[key] = load_w(*key)

                        ptA = pt_pool.tile([128, 512], F32, tag="pt",
                                           name="ptA")
                        ptB = None
                        if si == 0:
                            # the ap pool is idle during seg0 (no attention)
                            ptB = ap_pool.tile([128, 512], F32, tag="ap",
                                               name="ptB")
                        for half in range(2):
                            k0 = 16 * half
                            for kt in range(k0, k0 + 16):
                                nc.tensor.matmul(ptA[:, :sw], wt[:, kt, :],
                                                 hst[:, kt, :sw],
                                                 start=(kt == 0),
                                                 stop=(kt == NKT - 1))
                                if ptB is not None:
                                    nc.tensor.matmul(
                                        ptB[:, :64], wt[:, kt, :],
                                        hs_sliv[:, kt, :],
                                        start=(kt == 0),
                                        stop=(kt == NKT - 1))
                            pump_slot(3)
                        if m < 2 * HPC:
                            evac_qk(m, e, ptA, sw, s0)
                            if ptB is not None:
                                qkb = tc_ev_pool.tile([128, 64], BF,
                                                      tag="qkb", bufs=2,
                                                      name="qkb")
                                nc.scalar.activation(
                                    out=qkb[:], in_=ptB[:, :64],
                                    func=ACT_ID,
                                    bias=bias_t[:, m:m + 1], scale=1.0)
                                rope(m, qkb, 512, 64)
                        else:
                            pend_tp.append(evac_v(m, e, ptA, si, sw))
                            if ptB is not None:
                                mv = m - 2 * HPC
                                nc.scalar.activation(
                                    out=vTb[1][mv][:, 0:64],
                                    in_=ptB[:, :64], func=ACT_ID,
                                    bias=bias_t[:, m:m + 1], scale=1.0)

                    for tp in pend_tp:
                        tp()
                    pend_tp = []

                    if si == 3:
                        # finish any remaining chunk-2 work before dense
                        while active:
                            active[0].pump_some(10**6)
                            active.pop(0)
                        # prefetch first dense weight groups (4-o tiles)
                        for og in range(2):
                            for ee in range(2):
                                wdt = wd_pool.tile([128, 4, HPC, 128], BF,
                                                   tag="wd", name="wdt")
                                base = (ee * 32 + 4 * og) * HPC * 128
                                nc.sync.dma_start(
                                    out=wdt[:],
                                    in_=wd_d[:, base:base + 4 * HPC * 128])
                                wd_loaded.append(((ee, og), wdt))

            # ---------------- dense phase ----------------
            # pass A: tokens 0..1536 (interleaved with attn chunk 3),
            # pass B: tokens 1536..2048 (needs chunk 3's attnT).
            DCH_A = [[(0, 512), (512, 576)], [(576, 1088), (1088, 1536)]]
            last_chunk = AttnChunk(3)
            wd_tiles = dict(wd_loaded)

            def load_wd(ee, og):
                t = wd_pool.tile([128, 4, HPC, 128], BF, tag="wd",
                                 name="wdt")
                base = (ee * 32 + 4 * og) * HPC * 128
                nc.sync.dma_start(
                    out=t[:], in_=wd_d[:, base:base + 4 * HPC * 128])
                return t

            def get_wd(ee, og):
                t = wd_tiles.get((ee, og))
                if t is None:
                    t = load_wd(ee, og)
                    wd_tiles[(ee, og)] = t
                return t

            with tc.tile_pool(name="oep", bufs=3) as oe_pool:
                nsub = 32
                usched = [len(last_chunk.units) // nsub +
                          (1 if i < len(last_chunk.units) % nsub else 0)
                          for i in range(nsub)]
                sub_i = 0
                ei = 0
                for og in range(8):
                    for ee in range(2):
                        get_wd(ee, og)
                    if og + 2 < 8:
                        for ee in range(2):
                            get_wd(ee, og + 2)
                    for oi in range(HPC):
                        o = 4 * og + oi
                        oe = oe_pool.tile([128, 1536], BF, tag="oe",
                                          name="oe")
                        for ee in range(2):
                            wdt = get_wd(ee, og)
                            pos = [pt_pool.tile([128, 512], F32, tag="pt",
                                                name="po")
                                   for _ in DCH_A[ee]]
                            for hh in range(HPC):
                                for ci, (t0, t1) in enumerate(DCH_A[ee]):
                                    nc.tensor.matmul(
                                        pos[ci][:, :t1 - t0],
                                        wdt[:, oi, hh, :],
                                        attnT[hh][:, t0:t1],
                                        start=(hh == 0),
                                        stop=(hh == HPC - 1))
                            for ci, (t0, t1) in enumerate(DCH_A[ee]):
                                if ei % 2 == 0:
                                    nc.scalar.activation(
                                        out=oe[:, t0:t1],
                                        in_=pos[ci][:, :t1 - t0],
                                        func=ACT_COPY, scale=1.0)
                                else:
                                    nc.vector.tensor_copy(
                                        oe[:, t0:t1], pos[ci][:, :t1 - t0])
                                ei += 1
                        nc.scalar.dma_start(
                            out=out_d[128 * o:128 * (o + 1), 0:1536],
                            in_=oe[:])
                        last_chunk.pump_some(usched[sub_i])
                        sub_i += 1
                last_chunk.pump_some(10**6)

                # pass B — expert-1 tiles were recycled by the ring; reload.
                # accumulators alternate between the pt and (now idle) ap
                # pools for 4-deep psum pipelining.
                wdB = {0: load_wd(1, 0), 1: load_wd(1, 1)}
                for og in range(8):
                    wdt = wdB.pop(og)
                    if og + 2 < 8:
                        wdB[og + 2] = load_wd(1, og + 2)
                    for oi in range(HPC):
                        o = 4 * og + oi
                        if oi % 2 == 0:
                            po = pt_pool.tile([128, 512], F32, tag="pt",
                                              name="poB")
                        else:
                            po = ap_pool.tile([128, 512], F32, tag="ap",
                                              name="poB")
                        for hh in range(HPC):
                            nc.tensor.matmul(po[:], wdt[:, oi, hh, :],
                                             attnT[hh][:, 1536:2048],
                                             start=(hh == 0),
                                             stop=(hh == HPC - 1))
                        oe2 = oe_pool.tile([128, 512], BF, tag="oe2",
                                           name="oe2")
                        if ei % 2 == 0:
                            nc.scalar.activation(out=oe2[:], in_=po[:],
                                                 func=ACT_COPY, scale=1.0)
                        else:
                            nc.vector.tensor_copy(oe2[:], po[:])
                        ei += 1
                        nc.sync.dma_start(
                            out=out_d[128 * o:128 * (o + 1), 1536:2048],
                            in_=oe2[:])

            ap_ctx.__exit__(None, None, None)
            pt_ctx.__exit__(None, None, None)
            ps_ctx.__exit__(None, None, None)
            atn_ctx.__exit__(None, None, None)
            wd_ctx.__exit__(None, None, None)
    nc.compile()
    return nc


def _prep_inputs(inputs):
    import ml_dtypes
    bf = ml_dtypes.bfloat16

    hs = np.asarray(inputs["hidden_states"], np.float32)
    cos = np.asarray(inputs["cos"], np.float32)
    sin = np.asarray(inputs["sin"], np.float32)
    vi = np.asarray(inputs["vision_indices"]).ravel()
    li = np.asarray(inputs["language_indices"]).ravel()
    assert vi.size == NV and np.array_equal(vi, np.arange(NV)) and \
        np.array_equal(li, np.arange(NV, S)), "unsupported index layout"

    # hs tiled [128, (kt, t)]
    hs_t = np.ascontiguousarray(
        hs.T.reshape(NKT, 128, S).transpose(1, 0, 2).reshape(128, NKT * S)
    ).astype(bf)

    cos_t = np.ascontiguousarray(cos.T).astype(bf)
    sin_t = np.ascontiguousarray(sin.T).astype(bf)

    rm = np.zeros((D, D), np.float32)
    for d in range(64):
        rm[d + 64, d] = -1.0
        rm[d, d + 64] = 1.0
    rm = rm.astype(bf)

    # mask[p, q'] = 1 iff p <= q' within a 128 diagonal block
    p = np.arange(128)[:, None]
    q = np.arange(128)[None, :]
    mask = (p <= q).astype(np.float32).astype(bf)

    ones = np.ones((128, 1), np.float32).astype(bf)

    b = np.asarray(inputs["vision_qkv_b"], np.float32)
    Wqkv = np.stack([np.asarray(inputs["vision_qkv_w"], np.float32),
                     np.asarray(inputs["lang_qkv_w"], np.float32)])
    Wd = np.stack([np.asarray(inputs["vision_dense_w"], np.float32),
                   np.asarray(inputs["lang_dense_w"], np.float32)])

    in_maps = []
    for rr in range(NCORES):
        q0 = 512 * rr
        cols = np.r_[q0:q0 + 512, HID + q0:HID + q0 + 512,
                     2 * HID + q0:2 * HID + q0 + 512]
        # w tiled [128, (e, m, kt, c)]
        wc = Wqkv[:, :, cols]                                # [2, 4096, 1536]
        w_t = np.ascontiguousarray(
            wc.reshape(2, NKT, 128, NM, 128)
              .transpose(2, 0, 3, 1, 4)
              .reshape(128, 2 * NM * NKT * 128)).astype(bf)
        # wd tiled [128, (e, o, hh, c)]
        wdc = Wd[:, q0:q0 + 512, :]                          # [2, 512, 4096]
        wd_t = np.ascontiguousarray(
            wdc.reshape(2, HPC, 128, 32, 128)
               .transpose(2, 0, 3, 1, 4)
               .reshape(128, 2 * 32 * HPC * 128)).astype(bf)
        bias_t = np.ascontiguousarray(
            b[cols].reshape(NM, 128).T).astype(np.float32)
        in_maps.append({
            "hs": hs_t, "w": w_t, "wd": wd_t,
            "cos": cos_t, "sin": sin_t, "mask": mask, "rm": rm,
            "ones": ones, "onesr": np.ones((1, 128), np.float32).astype(bf),
            "bias": bias_t,
        })
    return in_maps


def kernel(**inputs):
    if "nc" not in _CACHE:
        _CACHE["nc"] = _build()
    nc = _CACHE["nc"]
    in_maps = _prep_inputs(inputs)
    res = run_bass_kernel_spmd(nc, in_maps, list(range(NCORES)),
                               **_CACHE.get("run_kwargs", {}))
    _CACHE["last_results"] = res
    out = np.zeros((HID, S), np.float32)
    for r in range(NCORES):
        out += res.results[r]["outT"].astype(np.float32)
    return np.ascontiguousarray(out.T)


# revision 34
# speedup vs baseline: 1.1079x; 1.0071x over previous
"""Trainium2 Bass kernel for modality-routed (CogVLM-style) attention, v3.

Contract: kernel(**inputs) takes FULL unsharded numpy inputs (as produced by
the reference's setup_inputs) and returns the FULL [2048, 4096] fp32 output.

Sharding: tensor-parallel over heads. Core r owns heads 4r..4r+3:
  - qkv weights column-sharded; q/k computed in transposed [dim, token]
    orientation via stationary weight tiles; v computed the same way then
    flipped to natural [token, dim] per 128-token tile with DMA transposes.
  - dense weights row-sharded; each core emits a partial output in transposed
    [4096, 2048] orientation, summed + transposed on the host.

v3 structural changes vs v2 (driven by the 880us trace):
  - attention is emitted at keytile granularity and INTERLEAVED between the
    next segment's QKV half-m-blocks, so exp (ACT) latency hides behind QKV
    matmuls instead of stalling the PE (attn windows were 45% PE-idle).
  - diagonal keytiles use restricted query widths (512/384/256/128) plus a
    single 128x128 triangular mask, cutting masked-out score/PV/exp work.
  - softmax normalization: ones-matmul partition sum -> reciprocal_approx_fast
    (vector.reciprocal was 3.3us per call) -> gpsimd partition_broadcast
    (replaces a DRAM bounce round-trip per head).
  - seg0's 64-token vision sliver shares LDWEIGHTS with the 512-token chunk
    (kt-major loop, two PSUM accumulators) instead of paying a LDW-bound pass.
  - DMAs are batched (1 per weight tile / segment / output row-block) and
    split across both HWDGE rings (plain on sync, transposes on scalar);
    the sync ring carried 496 serialized DMAs before.
  - dense runs chunk-major in two passes: tokens 0..1536 interleave with the
    last attention chunk, tokens 1536..2048 follow it; dense weights stream
    in 4-o-group tiles and output rows batch into 2 DMAs per row-block.
"""

import sys

import numpy as np

if "/opt/trn_rl_repo" not in sys.path:
    sys.path.insert(0, "/opt/trn_rl_repo")

import concourse.bass as bass  # noqa: E402,F401
import concourse.tile as tile  # noqa: E402
from concourse import bacc, mybir  # noqa: E402
from concourse.bass_utils import run_bass_kernel_spmd  # noqa: E402

S = 2048
HID = 4096
H = 32
D = 128
NCORES = 8
HPC = H // NCORES          # heads per core = 4
NV = 576                   # vision tokens occupy rows [0, NV)
NKT = HID // 128           # 32 K-tiles
NM = 3 * HPC               # 12 m-blocks (4 q, 4 k, 4 v)
QKSCALE = 1.0 / float(np.sqrt(D))

F32 = mybir.dt.float32
BF = mybir.dt.bfloat16
ACT_ID = None  # set in _build
ACT_COPY = None
ACT_EXP = None

# token segments (start, end, expert); segment si pairs with attn chunk si-1
SEGS = [(0, 576, 0), (576, 1088, 1), (1088, 1600, 1), (1600, 2048, 1)]
# v blocks (m 8..11) first so v tiles transpose early; q/k follow
M_ORDER = [8, 9, 10, 11, 0, 1, 2, 3, 4, 5, 6, 7]

_CACHE = {}


def _build():
    global ACT_ID, ACT_COPY, ACT_EXP
    ACT_ID = mybir.ActivationFunctionType.Identity
    ACT_COPY = mybir.ActivationFunctionType.Copy
    ACT_EXP = mybir.ActivationFunctionType.Exp

    nc = bacc.Bacc("TRN2", target_bir_lowering=False, debug=False,
                   num_devices=NCORES)
    dti = nc.dram_tensor
    hs_d = dti("hs", [128, NKT * S], BF, kind="ExternalInput").ap()
    w_d = dti("w", [128, 2 * NM * NKT * 128], BF, kind="ExternalInput").ap()
    wd_d = dti("wd", [128, 2 * 32 * HPC * 128], BF, kind="ExternalInput").ap()
    cos_d = dti("cos", [128, S], BF, kind="ExternalInput").ap()
    sin_d = dti("sin", [128, S], BF, kind="ExternalInput").ap()
    mask_d = dti("mask", [128, 128], BF, kind="ExternalInput").ap()
    rm_d = dti("rm", [D, D], BF, kind="ExternalInput").ap()
    ones_d = dti("ones", [128, 1], BF, kind="ExternalInput").ap()
    onesr_d = dti("onesr", [1, 128], BF, kind="ExternalInput").ap()
    bias_d = dti("bias", [128, NM], F32, kind="ExternalInput").ap()
    out_d = dti("outT", [HID, S], BF, kind="ExternalOutput").ap()

    hs3 = hs_d.tensor.ap().rearrange("p (k t) -> p k t", k=NKT)

    with tile.TileContext(nc) as tc:
        with tc.tile_pool(name="glob", bufs=1) as glob:
            cos_t = glob.tile([128, S], BF)
            sin_t = glob.tile([128, S], BF)
            mask_t = glob.tile([128, 128], BF)
            rm_t = glob.tile([D, D], BF)
            ones_t = glob.tile([128, 1], BF)
            onesr_t = glob.tile([1, 128], BF)
            bias_t = glob.tile([128, NM], F32)
            hs_sliv = glob.tile([128, NKT, 64], BF)

            qT = [glob.tile([128, S], BF, name=f"qT{h}") for h in range(HPC)]
            kT = [glob.tile([128, S], BF, name=f"kT{h}") for h in range(HPC)]
            v_sb = [glob.tile([128, 16, 128], BF, name=f"v{h}")
                    for h in range(HPC)]
            vTb = {b: [glob.tile([128, 128], BF, name=f"vb{b}{h}")
                       for h in range(HPC)] for b in (1, 2, 3)}
            attnT = [glob.tile([128, S], BF, name=f"attnT{h}")
                     for h in range(HPC)]

            # dense weights + attn pools outlive the QKV-only pools
            wd_ctx = tc.tile_pool(name="wdp", bufs=6)
            wd_pool = wd_ctx.__enter__()
            atn_ctx = tc.tile_pool(name="atn", bufs=1)
            atn_pool = atn_ctx.__enter__()
            ps_ctx = tc.tile_pool(name="ps512", bufs=4, space="PSUM")
            ps512 = ps_ctx.__enter__()
            pt_ctx = tc.tile_pool(name="ptp", bufs=2, space="PSUM")
            pt_pool = pt_ctx.__enter__()
            ap_ctx = tc.tile_pool(name="app", bufs=2, space="PSUM")
            ap_pool = ap_ctx.__enter__()

            # ---------------- attention chunk machinery ----------------
            # unit = one keytile of one head: score MM -> exp -> (mask) ->
            # acc -> PV MM. pend keeps a 3-deep score->PV pipeline.
            class AttnChunk:
                def __init__(self, c):
                    self.c = c
                    self.q0 = 512 * c
                    self.units = []
                    for h in range(HPC):
                        for j in range(4):        # diagonal keytiles
                            self.units.append(
                                (h, 4 * c + j, 128 * j, True, j == 0,
                                 c == 0 and j == 3))
                        for jt in range(4 * c):   # off-diagonal keytiles
                            self.units.append(
                                (h, jt, 0, False, False, jt == 4 * c - 1))
                    self.pend = []
                    self.acc = {}
                    self.ap = {}
                    self.idx = 0

                def emit_unit(self, u):
                    (h, ktj, off, diag, first, last) = u
                    w = 512 - off
                    sc = ps512.tile([128, 512], F32, tag="u", name="sc")
                    nc.tensor.matmul(sc[:, off:512],
                                     kT[h][:, 128 * ktj:128 * (ktj + 1)],
                                     qT[h][:, self.q0 + off:self.q0 + 512],
                                     start=True, stop=True)
                    pb = atn_pool.tile([128, 512], BF, tag="pb", bufs=4,
                                       name="pb")
                    nc.scalar.activation(out=pb[:, off:512],
                                         in_=sc[:, off:512],
                                         func=ACT_EXP, scale=QKSCALE)
                    if diag:
                        nc.gpsimd.tensor_mul(pb[:, off:off + 128],
                                             pb[:, off:off + 128], mask_t[:])
                    if first:
                        acc = atn_pool.tile([128, 512], BF, tag="acc",
                                            bufs=2, name="acc")
                        self.acc[h] = acc
                        nc.vector.tensor_copy(acc[:], pb[:])
                    else:
                        acc = self.acc[h]
                        nc.vector.tensor_add(acc[:, off:], acc[:, off:],
                                             pb[:, off:])
                    self.pend.append((h, ktj, off, first, last, pb))
                    if len(self.pend) > 2:
                        self.flush_one()

                def flush_one(self):
                    (h, ktj, off, first, last, pb) = self.pend.pop(0)
                    if first:
                        self.ap[h] = ap_pool.tile([128, 512], F32, tag="ap",
                                                  name="ap")
                    nc.tensor.matmul(self.ap[h][:, off:512],
                                     v_sb[h][:, ktj, :], pb[:, off:512],
                                     start=first, stop=last)
                    if last:
                        self.finish(h)

                def finish(self, h):
                    sp = ps512.tile([128, 512], F32, tag="u", name="sp")
                    nc.tensor.matmul(sp[0:1, :], ones_t[:], self.acc[h][:],
                                     start=True, stop=True)
                    rcf = atn_pool.tile([1, 512], F32, tag="rcf", bufs=2,
                                        name="rcf")
                    nc.vector.reciprocal_approx_fast(out=rcf[:],
                                                     in_=sp[0:1, :])
                    rcb = atn_pool.tile([1, 512], BF, tag="rcb", bufs=2,
                                        name="rcb")
                    nc.vector.tensor_copy(rcb[:], rcf[:])
                    # broadcast rcb across partitions: K=1 outer product
                    rbp = ps512.tile([128, 512], F32, tag="u", name="rbp")
                    nc.tensor.matmul(rbp[:], onesr_t[0:1, :], rcb[0:1, :],
                                     start=True, stop=True)
                    rb = atn_pool.tile([128, 512], BF, tag="rb", bufs=2,
                                       name="rb")
                    nc.vector.tensor_copy(rb[:], rbp[:])
                    nc.vector.tensor_mul(
                        attnT[h][:, self.q0:self.q0 + 512],
                        self.ap[h][:], rb[:])

                def pump_some(self, n):
                    used = 0
                    while used < n:
                        if self.idx < len(self.units):
                            self.emit_unit(self.units[self.idx])
                            self.idx += 1
                            used += 1
                        elif self.pend:
                            self.flush_one()
                            used += 1
                        else:
                            break
                    return used

                def done(self):
                    return self.idx >= len(self.units) and not self.pend

            # ---------------- QKV emission ----------------
            def load_w(e, m):
                wt = tc_w_pool.tile([128, NKT, 128], BF, tag="w", name="wt")
                base = (e * NM + m) * NKT * 128
                # two halves so the m-block's first matmuls start early
                nc.sync.dma_start(out=wt[:, 0:16, :],
                                  in_=w_d[:, base:base + 16 * 128])
                nc.sync.dma_start(out=wt[:, 16:32, :],
                                  in_=w_d[:, base + 16 * 128:base + NKT * 128])
                return wt

            def rope(m, qk_sb, c0, w):
                rot = ps512.tile([128, 512], F32, tag="u", name="rot")
                nc.tensor.matmul(rot[:, :w], rm_t[:], qk_sb[:, :w],
                                 start=True, stop=True)
                prod = tc_ev_pool.tile([128, 512], BF, tag="prod", bufs=2,
                                       name="prod")
                nc.vector.tensor_mul(prod[:, :w], qk_sb[:, :w],
                                     cos_t[:, c0:c0 + w])
                rp = tc_ev_pool.tile([128, 512], BF, tag="rp", bufs=2,
                                     name="rp")
                nc.vector.tensor_mul(rp[:, :w], rot[:, :w],
                                     sin_t[:, c0:c0 + w])
                tgt = qT[m] if m < HPC else kT[m - HPC]
                nc.vector.tensor_add(tgt[:, c0:c0 + w], prod[:, :w],
                                     rp[:, :w])

            def evac_qk(m, e, pt, w, c0):
                qk_sb = tc_ev_pool.tile([128, 512], BF, tag="qksb", bufs=2,
                                        name="qk_sb")
                if e == 0:
                    nc.scalar.activation(out=qk_sb[:, :w], in_=pt[:, :w],
                                         func=ACT_ID,
                                         bias=bias_t[:, m:m + 1], scale=1.0)
                else:
                    nc.scalar.activation(out=qk_sb[:, :w], in_=pt[:, :w],
                                         func=ACT_COPY, scale=1.0)
                rope(m, qk_sb, c0, w)

            def evac_v(m, e, pt, si, w):
                # seg si covers tokens [s0, s0+w); for si>0, s0 % 128 == 64.
                mv = m - 2 * HPC
                o1 = 64 if si else 0
                stg = tc_ev_pool.tile([128, 576], BF, tag="vstg", bufs=3,
                                      name="stg")
                if e == 0:
                    nc.scalar.activation(out=stg[:, o1:o1 + w],
                                         in_=pt[:, :w], func=ACT_ID,
                                         bias=bias_t[:, m:m + 1], scale=1.0)
                else:
                    nc.scalar.activation(out=stg[:, o1:o1 + w],
                                         in_=pt[:, :w], func=ACT_COPY,
                                         scale=1.0)
                # transposes are DEFERRED one m-block (returned as a closure)
                # so their input-ready waits never head-of-line block the
                # sync DMA FIFO in front of weight/hs streams.
                # split the 4 transposes across both HWDGE rings so neither
                # the weight stream (sync) nor the evac/exp queue (scalar)
                # eats the full burst
                if si == 0:
                    def tp():
                        for jt in range(4):
                            nc.sync.dma_start_transpose(
                                out=v_sb[mv][:, jt, :],
                                in_=stg[:, 128 * jt:128 * (jt + 1)])
                else:
                    b = si
                    nc.vector.tensor_copy(vTb[b][mv][:, 64:128],
                                          stg[:, 64:128])
                    if b + 1 <= 3 and w == 512:
                        nc.vector.tensor_copy(vTb[b + 1][mv][:, 0:64],
                                              stg[:, 576 - 64:576])

                    def tp():
                        nc.sync.dma_start_transpose(
                            out=v_sb[mv][:, 4 * b, :], in_=vTb[b][mv][:])
                        for i in range(3):
                            nc.sync.dma_start_transpose(
                                out=v_sb[mv][:, 4 * b + 1 + i, :],
                                in_=stg[:, 128 * (i + 1):128 * (i + 2)])
                return tp

            with tc.tile_pool(name="hsp", bufs=2) as hs_pool, \
                 tc.tile_pool(name="wp", bufs=2) as tc_w_pool, \
                 tc.tile_pool(name="evp", bufs=2) as tc_ev_pool:

                # seg0 streams first so the PE starts ASAP; constants load
                # behind them.
                # quarter-granularity first weight load: the very first
                # matmul only needs kt 0..7 resident
                wt0 = tc_w_pool.tile([128, NKT, 128], BF, tag="w", name="wt")
                b0 = (0 * NM + 8) * NKT * 128
                for q in range(4):
                    nc.sync.dma_start(
                        out=wt0[:, 8 * q:8 * (q + 1), :],
                        in_=w_d[:, b0 + q * 1024:b0 + (q + 1) * 1024])
                    if q == 0:
                        hst0 = hs_pool.tile([128, NKT, 512], BF, tag="hs",
                                            name="hst")
                        nc.sync.dma_start(out=hst0[:, 0:8, :],
                                          in_=hs3[:, 0:8, 0:512])
                        nc.sync.dma_start(out=hs_sliv[:, 0:8, :],
                                          in_=hs3[:, 0:8, 512:576])
                wpre = {(0, 8): wt0}
                for kg in range(8, NKT, 8):
                    nc.sync.dma_start(out=hst0[:, kg:kg + 8, :],
                                      in_=hs3[:, kg:kg + 8, 0:512])
                nc.sync.dma_start(out=hs_sliv[:, 8:, :],
                                  in_=hs3[:, 8:, 512:576])
                nc.sync.dma_start(out=bias_t[:], in_=bias_d[:])
                nc.sync.dma_start(out=rm_t[:], in_=rm_d[:])
                wpre[(0, 9)] = load_w(0, 9)
                nc.sync.dma_start(out=cos_t[:, :576], in_=cos_d[:, :576])
                nc.sync.dma_start(out=sin_t[:, :576], in_=sin_d[:, :576])
                nc.sync.dma_start(out=ones_t[:], in_=ones_d[:])
                nc.sync.dma_start(out=onesr_t[:], in_=onesr_d[:])
                nc.sync.dma_start(out=mask_t[:], in_=mask_d[:])
                nc.sync.dma_start(out=cos_t[:, 576:], in_=cos_d[:, 576:])
                nc.sync.dma_start(out=sin_t[:, 576:], in_=sin_d[:, 576:])
                # warm the exp table set early (one-element activation)
                exp_warm = tc_ev_pool.tile([1, 1], F32, tag="ew", bufs=1,
                                           name="exp_warm")
                nc.scalar.activation(out=exp_warm[:], in_=bias_t[0:1, 0:1],
                                     func=ACT_EXP, scale=1.0)

                hs_cur = hst0
                active = []          # attention chunks with remaining work
                pend_tp = []         # deferred v-transpose closures
                wd_loaded = []

                def pump_slot(n):
                    while n > 0 and active:
                        used = active[0].pump_some(n)
                        if active[0].done():
                            active.pop(0)
                        if used == 0 and not active:
                            break
                        n -= used

                for si, (s0, s1, e) in enumerate(SEGS):
                    sw = s1 - s0 if si else 512
                    hst = hs_cur
                    if si + 1 < len(SEGS):
                        n0, n1, _ = SEGS[si + 1]
                        hs_cur = hs_pool.tile([128, NKT, 512], BF, tag="hs",
                                              name="hst")
                    if si >= 1:
                        active.append(AttnChunk(si - 1))

                    for mi, m in enumerate(M_ORDER):
                        # flush deferred transposes from the previous block
                        for tp in pend_tp:
                            tp()
                        pend_tp = []
                        # spread next segment's hs prefetch into the q/k
                        # m-blocks (mi>=4): the v blocks' transposes already
                        # load the sync ring
                        if si + 1 < len(SEGS) and mi in (5, 7, 9, 11):
                            kg = 8 * ((mi - 5) // 2)
                            nc.sync.dma_start(
                                out=hs_cur[:, kg:kg + 8, :n1 - n0],
                                in_=hs3[:, kg:kg + 8, n0:n1])
                        wt = wpre.pop((e, m), None)
                        if wt is None:
                            wt = load_w(e, m)
                        # prefetch 1 m-block ahead (same or next segment)
                        pf = mi + 1
                        if pf < len(M_ORDER):
                            key = (e, M_ORDER[pf])
                            if key not in wpre:
                                wpre[key] = load_w(*key)
                        elif si + 1 < len(SEGS):
                            key = (SEGS[si + 1][2], M_ORDER[pf - len(M_ORDER)])
                            if key not in wpre:
                                wpre[key] = load_w(*key)

                        ptA = pt_pool.tile([128, 512], F32, tag="pt",
                                           name="ptA")
                        ptB = None
                        if si == 0:
                            # the ap pool is idle during seg0 (no attention)
                            ptB = ap_pool.tile([128, 512], F32, tag="ap",
                                               name="ptB")
                        for half in range(2):
                            k0 = 16 * half
                            for kt in range(k0, k0 + 16):
                                nc.tensor.matmul(ptA[:, :sw], wt[:, kt, :],
                                                 hst[:, kt, :sw],
                                                 start=(kt == 0),
                                                 stop=(kt == NKT - 1))
                                if ptB is not None:
                                    nc.tensor.matmul(
                                        ptB[:, :64], wt[:, kt, :],
                                        hs_sliv[:, kt, :],
                                        start=(kt == 0),
                                        stop=(kt == NKT - 1))
                            pump_slot(3)
                        if m < 2 * HPC:
                            evac_qk(m, e, ptA, sw, s0)
                            if ptB is not None:
                                qkb = tc_ev_pool.tile([128, 64], BF,
                                                      tag="qkb", bufs=2,
                                                      name="qkb")
                                nc.scalar.activation(
                                    out=qkb[:], in_=ptB[:, :64],
                                    func=ACT_ID,
                                    bias=bias_t[:, m:m + 1], scale=1.0)
                                rope(m, qkb, 512, 64)
                        else:
                            pend_tp.append(evac_v(m, e, ptA, si, sw))
                            if ptB is not None:
                                mv = m - 2 * HPC
                                nc.scalar.activation(
                                    out=vTb[1][mv][:, 0:64],
                                    in_=ptB[:, :64], func=ACT_ID,
                                    bias=bias_t[:, m:m + 1], scale=1.0)

                    for tp in pend_tp:
                        tp()
                    pend_tp = []

                    if si == 3:
                        # finish any remaining chunk-2 work before dense
                        while active:
                            active[0].pump_some(10**6)
                            active.pop(0)
                        # prefetch first dense weight groups (4-o tiles)
                        for og in range(2):
                            for ee in range(2):
                                wdt = wd_pool.tile([128, 4, HPC, 128], BF,
                                                   tag="wd", name="wdt")
                                base = (ee * 32 + 4 * og) * HPC * 128
                                nc.sync.dma_start(
                                    out=wdt[:],
                                    in_=wd_d[:, base:base + 4 * HPC * 128])
                                wd_loaded.append(((ee, og), wdt))

            # ---------------- dense phase ----------------
            # pass A: tokens 0..1536 (interleaved with attn chunk 3),
            # pass B: tokens 1536..2048 (needs chunk 3's attnT).
            DCH_A = [[(0, 512), (512, 576)], [(576, 1088), (1088, 1536)]]
            last_chunk = AttnChunk(3)
            wd_tiles = dict(wd_loaded)

            def load_wd(ee, og):
                t = wd_pool.tile([128, 4, HPC, 128], BF, tag="wd",
                                 name="wdt")
                base = (ee * 32 + 4 * og) * HPC * 128
                nc.sync.dma_start(
                    out=t[:], in_=wd_d[:, base:base + 4 * HPC * 128])
                return t

            def get_wd(ee, og):
                t = wd_tiles.get((ee, og))
                if t is None:
                    t = load_wd(ee, og)
                    wd_tiles[(ee, og)] = t
                return t

            with tc.tile_pool(name="oep", bufs=3) as oe_pool:
                nsub = 32
                usched = [len(last_chunk.units) // nsub +
                          (1 if i < len(last_chunk.units) % nsub else 0)
                          for i in range(nsub)]
                sub_i = 0
                ei = 0
                for og in range(8):
                    for ee in range(2):
                        get_wd(ee, og)
                    if og + 2 < 8:
                        for ee in range(2):
                            get_wd(ee, og + 2)
                    for oi in range(HPC):
                        o = 4 * og + oi
                        oe = oe_pool.tile([128, 1536], BF, tag="oe",
                                          name="oe")
                        for ee in range(2):
                            wdt = get_wd(ee, og)
                            pos = [pt_pool.tile([128, 512], F32, tag="pt",
                                                name="po")
                                   for _ in DCH_A[ee]]
                            for hh in range(HPC):
                                for ci, (t0, t1) in enumerate(DCH_A[ee]):
                                    nc.tensor.matmul(
                                        pos[ci][:, :t1 - t0],
                                        wdt[:, oi, hh, :],
                                        attnT[hh][:, t0:t1],
                                        start=(hh == 0),
                                        stop=(hh == HPC - 1))
                            for ci, (t0, t1) in enumerate(DCH_A[ee]):
                                if ei % 2 == 0:
                                    nc.scalar.activation(
                                        out=oe[:, t0:t1],
                                        in_=pos[ci][:, :t1 - t0],
                                        func=ACT_COPY, scale=1.0)
                                else:
                                    nc.vector.tensor_copy(
                                        oe[:, t0:t1], pos[ci][:, :t1 - t0])
                                ei += 1
                        nc.scalar.dma_start(
                            out=out_d[128 * o:128 * (o + 1), 0:1536],
                            in_=oe[:])
                        last_chunk.pump_some(usched[sub_i])
                        sub_i += 1
                last_chunk.pump_some(10**6)

                # pass B — expert-1 tiles were recycled by the ring; reload.
                # accumulators alternate between the pt and (now idle) ap
                # pools for 4-deep psum pipelining.
                wdB = {0: load_wd(1, 0), 1: load_wd(1, 1)}
                for og in range(8):
                    wdt = wdB.pop(og)
                    if og + 2 < 8:
                        wdB[og + 2] = load_wd(1, og + 2)
                    for oi in range(HPC):
                        o = 4 * og + oi
                        if oi % 2 == 0:
                            po = pt_pool.tile([128, 512], F32, tag="pt",
                                              name="poB")
                        else:
                            po = ap_pool.tile([128, 512], F32, tag="ap",
                                              name="poB")
                        for hh in range(HPC):
                            nc.tensor.matmul(po[:], wdt[:, oi, hh, :],
                                             attnT[hh][:, 1536:2048],
                                             start=(hh == 0),
                                             stop=(hh == HPC - 1))
                        oe2 = oe_pool.tile([128, 512], BF, tag="oe2",
                                           name="oe2")
                        if ei % 2 == 0:
                            nc.scalar.activation(out=oe2[:], in_=po[:],
                                                 func=ACT_COPY, scale=1.0)
                        else:
                            nc.vector.tensor_copy(oe2[:], po[:])
                        ei += 1
                        nc.sync.dma_start(
                            out=out_d[128 * o:128 * (o + 1), 1536:2048],
                            in_=oe2[:])

            ap_ctx.__exit__(None, None, None)
            pt_ctx.__exit__(None, None, None)
            ps_ctx.__exit__(None, None, None)
            atn_ctx.__exit__(None, None, None)
            wd_ctx.__exit__(None, None, None)
    nc.compile()
    return nc


def _prep_inputs(inputs):
    import ml_dtypes
    bf = ml_dtypes.bfloat16

    hs = np.asarray(inputs["hidden_states"], np.float32)
    cos = np.asarray(inputs["cos"], np.float32)
    sin = np.asarray(inputs["sin"], np.float32)
    vi = np.asarray(inputs["vision_indices"]).ravel()
    li = np.asarray(inputs["language_indices"]).ravel()
    assert vi.size == NV and np.array_equal(vi, np.arange(NV)) and \
        np.array_equal(li, np.arange(NV, S)), "unsupported index layout"

    # hs tiled [128, (kt, t)]
    hs_t = np.ascontiguousarray(
        hs.T.reshape(NKT, 128, S).transpose(1, 0, 2).reshape(128, NKT * S)
    ).astype(bf)

    cos_t = np.ascontiguousarray(cos.T).astype(bf)
    sin_t = np.ascontiguousarray(sin.T).astype(bf)

    rm = np.zeros((D, D), np.float32)
    for d in range(64):
        rm[d + 64, d] = -1.0
        rm[d, d + 64] = 1.0
    rm = rm.astype(bf)

    # mask[p, q'] = 1 iff p <= q' within a 128 diagonal block
    p = np.arange(128)[:, None]
    q = np.arange(128)[None, :]
    mask = (p <= q).astype(np.float32).astype(bf)

    ones = np.ones((128, 1), np.float32).astype(bf)

    b = np.asarray(inputs["vision_qkv_b"], np.float32)
    Wqkv = np.stack([np.asarray(inputs["vision_qkv_w"], np.float32),
                     np.asarray(inputs["lang_qkv_w"], np.float32)])
    Wd = np.stack([np.asarray(inputs["vision_dense_w"], np.float32),
                   np.asarray(inputs["lang_dense_w"], np.float32)])

    in_maps = []
    for rr in range(NCORES):
        q0 = 512 * rr
        cols = np.r_[q0:q0 + 512, HID + q0:HID + q0 + 512,
                     2 * HID + q0:2 * HID + q0 + 512]
        # w tiled [128, (e, m, kt, c)]
        wc = Wqkv[:, :, cols]                                # [2, 4096, 1536]
        w_t = np.ascontiguousarray(
            wc.reshape(2, NKT, 128, NM, 128)
              .transpose(2, 0, 3, 1, 4)
              .reshape(128, 2 * NM * NKT * 128)).astype(bf)
        # wd tiled [128, (e, o, hh, c)]
        wdc = Wd[:, q0:q0 + 512, :]                          # [2, 512, 4096]
        wd_t = np.ascontiguousarray(
            wdc.reshape(2, HPC, 128, 32, 128)
               .transpose(2, 0, 3, 1, 4)
               .reshape(128, 2 * 32 * HPC * 128)).astype(bf)
        bias_t = np.ascontiguousarray(
            b[cols].reshape(NM, 128).T).astype(np.float32)
        in_maps.append({
            "hs": hs_t, "w": w_t, "wd": wd_t,
            "cos": cos_t, "sin": sin_t, "mask": mask, "rm": rm,
            "ones": ones, "onesr": np.ones((1, 128), np.float32).astype(bf),
            "bias": bias_t,
        })
    return in_maps


def kernel(**inputs):
    if "nc" not in _CACHE:
        _CACHE["nc"] = _build()
    nc = _CACHE["nc"]
    in_maps = _prep_inputs(inputs)
    res = run_bass_kernel_spmd(nc, in_maps, list(range(NCORES)),
                               **_CACHE.get("run_kwargs", {}))
    _CACHE["last_results"] = res
    out = np.zeros((HID, S), np.float32)
    for r in range(NCORES):
        out += res.results[r]["outT"].astype(np.float32)
    return np.ascontiguousarray(out.T)


# revision 35
# speedup vs baseline: 1.1305x; 1.0204x over previous
"""Trainium2 Bass kernel for modality-routed (CogVLM-style) attention, v3.

Contract: kernel(**inputs) takes FULL unsharded numpy inputs (as produced by
the reference's setup_inputs) and returns the FULL [2048, 4096] fp32 output.

Sharding: tensor-parallel over heads. Core r owns heads 4r..4r+3:
  - qkv weights column-sharded; q/k computed in transposed [dim, token]
    orientation via stationary weight tiles; v computed the same way then
    flipped to natural [token, dim] per 128-token tile with DMA transposes.
  - dense weights row-sharded; each core emits a partial output in transposed
    [4096, 2048] orientation, summed + transposed on the host.

v3 structural changes vs v2 (driven by the 880us trace):
  - attention is emitted at keytile granularity and INTERLEAVED between the
    next segment's QKV half-m-blocks, so exp (ACT) latency hides behind QKV
    matmuls instead of stalling the PE (attn windows were 45% PE-idle).
  - diagonal keytiles use restricted query widths (512/384/256/128) plus a
    single 128x128 triangular mask, cutting masked-out score/PV/exp work.
  - softmax normalization: ones-matmul partition sum -> reciprocal_approx_fast
    (vector.reciprocal was 3.3us per call) -> gpsimd partition_broadcast
    (replaces a DRAM bounce round-trip per head).
  - seg0's 64-token vision sliver shares LDWEIGHTS with the 512-token chunk
    (kt-major loop, two PSUM accumulators) instead of paying a LDW-bound pass.
  - DMAs are batched (1 per weight tile / segment / output row-block) and
    split across both HWDGE rings (plain on sync, transposes on scalar);
    the sync ring carried 496 serialized DMAs before.
  - dense runs chunk-major in two passes: tokens 0..1536 interleave with the
    last attention chunk, tokens 1536..2048 follow it; dense weights stream
    in 4-o-group tiles and output rows batch into 2 DMAs per row-block.
"""

import sys

import numpy as np

if "/opt/trn_rl_repo" not in sys.path:
    sys.path.insert(0, "/opt/trn_rl_repo")

import concourse.bass as bass  # noqa: E402,F401
import concourse.tile as tile  # noqa: E402
from concourse import bacc, mybir  # noqa: E402
from concourse.bass_utils import run_bass_kernel_spmd  # noqa: E402

S = 2048
HID = 4096
H = 32
D = 128
NCORES = 8
HPC = H // NCORES          # heads per core = 4
NV = 576                   # vision tokens occupy rows [0, NV)
NKT = HID // 128           # 32 K-tiles
NM = 3 * HPC               # 12 m-blocks (4 q, 4 k, 4 v)
QKSCALE = 1.0 / float(np.sqrt(D))

F32 = mybir.dt.float32
BF = mybir.dt.bfloat16
ACT_ID = None  # set in _build
ACT_COPY = None
ACT_EXP = None

# token segments (start, end, expert); segment si pairs with attn chunk si-1
SEGS = [(0, 576, 0), (576, 1088, 1), (1088, 1600, 1), (1600, 2048, 1)]
# v blocks (m 8..11) first so v tiles transpose early; q/k follow
M_ORDER = [8, 9, 10, 11, 0, 1, 2, 3, 4, 5, 6, 7]

_CACHE = {}


def _build():
    global ACT_ID, ACT_COPY, ACT_EXP
    ACT_ID = mybir.ActivationFunctionType.Identity
    ACT_COPY = mybir.ActivationFunctionType.Copy
    ACT_EXP = mybir.ActivationFunctionType.Exp

    nc = bacc.Bacc("TRN2", target_bir_lowering=False, debug=False,
                   num_devices=NCORES)
    dti = nc.dram_tensor
    hs_d = dti("hs", [128, NKT * S], BF, kind="ExternalInput").ap()
    w_d = dti("w", [128, 2 * NM * NKT * 128], BF, kind="ExternalInput").ap()
    wd_d = dti("wd", [128, 2 * 32 * HPC * 128], BF, kind="ExternalInput").ap()
    cos_d = dti("cos", [128, S], BF, kind="ExternalInput").ap()
    sin_d = dti("sin", [128, S], BF, kind="ExternalInput").ap()
    mask_d = dti("mask", [128, 128], BF, kind="ExternalInput").ap()
    rm_d = dti("rm", [D, D], BF, kind="ExternalInput").ap()
    ones_d = dti("ones", [128, 1], BF, kind="ExternalInput").ap()
    onesr_d = dti("onesr", [1, 128], BF, kind="ExternalInput").ap()
    bias_d = dti("bias", [128, NM], F32, kind="ExternalInput").ap()
    out_d = dti("outT", [HID, S], BF, kind="ExternalOutput").ap()

    hs3 = hs_d.tensor.ap().rearrange("p (k t) -> p k t", k=NKT)

    with tile.TileContext(nc) as tc:
        with tc.tile_pool(name="glob", bufs=1) as glob:
            cos_t = glob.tile([128, S], BF)
            sin_t = glob.tile([128, S], BF)
            mask_t = glob.tile([128, 128], BF)
            rm_t = glob.tile([D, D], BF)
            ones_t = glob.tile([128, 1], BF)
            onesr_t = glob.tile([1, 128], BF)
            bias_t = glob.tile([128, NM], F32)
            hs_sliv = glob.tile([128, NKT, 64], BF)

            qT = [glob.tile([128, S], BF, name=f"qT{h}") for h in range(HPC)]
            kT = [glob.tile([128, S], BF, name=f"kT{h}") for h in range(HPC)]
            v_sb = [glob.tile([128, 16, 128], BF, name=f"v{h}")
                    for h in range(HPC)]
            vTb = {b: [glob.tile([128, 128], BF, name=f"vb{b}{h}")
                       for h in range(HPC)] for b in (1, 2, 3)}
            attnT = [glob.tile([128, S], BF, name=f"attnT{h}")
                     for h in range(HPC)]

            # dense weights + attn pools outlive the QKV-only pools
            wd_ctx = tc.tile_pool(name="wdp", bufs=6)
            wd_pool = wd_ctx.__enter__()
            atn_ctx = tc.tile_pool(name="atn", bufs=1)
            atn_pool = atn_ctx.__enter__()
            ps_ctx = tc.tile_pool(name="ps512", bufs=4, space="PSUM")
            ps512 = ps_ctx.__enter__()
            pt_ctx = tc.tile_pool(name="ptp", bufs=2, space="PSUM")
            pt_pool = pt_ctx.__enter__()
            ap_ctx = tc.tile_pool(name="app", bufs=2, space="PSUM")
            ap_pool = ap_ctx.__enter__()

            # ---------------- attention chunk machinery ----------------
            # unit = one keytile of one head: score MM -> exp -> (mask) ->
            # acc -> PV MM. pend keeps a 3-deep score->PV pipeline.
            class AttnChunk:
                def __init__(self, c):
                    self.c = c
                    self.q0 = 512 * c
                    self.units = []
                    for h in range(HPC):
                        for j in range(4):        # diagonal keytiles
                            self.units.append(
                                (h, 4 * c + j, 128 * j, True, j == 0,
                                 c == 0 and j == 3))
                        for jt in range(4 * c):   # off-diagonal keytiles
                            self.units.append(
                                (h, jt, 0, False, False, jt == 4 * c - 1))
                    self.pend = []
                    self.acc = {}
                    self.ap = {}
                    self.idx = 0

                def emit_unit(self, u):
                    (h, ktj, off, diag, first, last) = u
                    w = 512 - off
                    sc = ps512.tile([128, 512], F32, tag="u", name="sc")
                    nc.tensor.matmul(sc[:, off:512],
                                     kT[h][:, 128 * ktj:128 * (ktj + 1)],
                                     qT[h][:, self.q0 + off:self.q0 + 512],
                                     start=True, stop=True)
                    pb = atn_pool.tile([128, 512], BF, tag="pb", bufs=4,
                                       name="pb")
                    nc.scalar.activation(out=pb[:, off:512],
                                         in_=sc[:, off:512],
                                         func=ACT_EXP, scale=QKSCALE)
                    if diag:
                        nc.gpsimd.tensor_mul(pb[:, off:off + 128],
                                             pb[:, off:off + 128], mask_t[:])
                    if first:
                        acc = atn_pool.tile([128, 512], BF, tag="acc",
                                            bufs=2, name="acc")
                        self.acc[h] = acc
                        nc.vector.tensor_copy(acc[:], pb[:])
                    else:
                        acc = self.acc[h]
                        nc.vector.tensor_add(acc[:, off:], acc[:, off:],
                                             pb[:, off:])
                    self.pend.append((h, ktj, off, first, last, pb))
                    if len(self.pend) > 2:
                        self.flush_one()

                def flush_one(self):
                    (h, ktj, off, first, last, pb) = self.pend.pop(0)
                    if first:
                        self.ap[h] = ap_pool.tile([128, 512], F32, tag="ap",
                                                  name="ap")
                    nc.tensor.matmul(self.ap[h][:, off:512],
                                     v_sb[h][:, ktj, :], pb[:, off:512],
                                     start=first, stop=last)
                    if last:
                        self.finish(h)

                def finish(self, h):
                    sp = ps512.tile([128, 512], F32, tag="u", name="sp")
                    nc.tensor.matmul(sp[0:1, :], ones_t[:], self.acc[h][:],
                                     start=True, stop=True)
                    rcf = atn_pool.tile([1, 512], F32, tag="rcf", bufs=2,
                                        name="rcf")
                    nc.vector.reciprocal_approx_fast(out=rcf[:],
                                                     in_=sp[0:1, :])
                    rcb = atn_pool.tile([1, 512], BF, tag="rcb", bufs=2,
                                        name="rcb")
                    nc.vector.tensor_copy(rcb[:], rcf[:])
                    # broadcast rcb across partitions: K=1 outer product
                    rbp = ps512.tile([128, 512], F32, tag="u", name="rbp")
                    nc.tensor.matmul(rbp[:], onesr_t[0:1, :], rcb[0:1, :],
                                     start=True, stop=True)
                    rb = atn_pool.tile([128, 512], BF, tag="rb", bufs=2,
                                       name="rb")
                    nc.vector.tensor_copy(rb[:], rbp[:])
                    nc.vector.tensor_mul(
                        attnT[h][:, self.q0:self.q0 + 512],
                        self.ap[h][:], rb[:])

                def pump_some(self, n):
                    used = 0
                    while used < n:
                        if self.idx < len(self.units):
                            self.emit_unit(self.units[self.idx])
                            self.idx += 1
                            used += 1
                        elif self.pend:
                            self.flush_one()
                            used += 1
                        else:
                            break
                    return used

                def done(self):
                    return self.idx >= len(self.units) and not self.pend

            # ---------------- QKV emission ----------------
            def load_w(e, m):
                wt = tc_w_pool.tile([128, NKT, 128], BF, tag="w", name="wt")
                base = (e * NM + m) * NKT * 128
                # two halves so the m-block's first matmuls start early
                nc.sync.dma_start(out=wt[:, 0:16, :],
                                  in_=w_d[:, base:base + 16 * 128])
                nc.sync.dma_start(out=wt[:, 16:32, :],
                                  in_=w_d[:, base + 16 * 128:base + NKT * 128])
                return wt

            def rope(m, qk_sb, c0, w):
                rot = ps512.tile([128, 512], F32, tag="u", name="rot")
                nc.tensor.matmul(rot[:, :w], rm_t[:], qk_sb[:, :w],
                                 start=True, stop=True)
                prod = tc_ev_pool.tile([128, 512], BF, tag="prod", bufs=2,
                                       name="prod")
                nc.vector.tensor_mul(prod[:, :w], qk_sb[:, :w],
                                     cos_t[:, c0:c0 + w])
                rp = tc_ev_pool.tile([128, 512], BF, tag="rp", bufs=2,
                                     name="rp")
                nc.vector.tensor_mul(rp[:, :w], rot[:, :w],
                                     sin_t[:, c0:c0 + w])
                tgt = qT[m] if m < HPC else kT[m - HPC]
                nc.vector.tensor_add(tgt[:, c0:c0 + w], prod[:, :w],
                                     rp[:, :w])

            def evac_qk(m, e, pt, w, c0):
                qk_sb = tc_ev_pool.tile([128, 512], BF, tag="qksb", bufs=2,
                                        name="qk_sb")
                if e == 0:
                    nc.scalar.activation(out=qk_sb[:, :w], in_=pt[:, :w],
                                         func=ACT_ID,
                                         bias=bias_t[:, m:m + 1], scale=1.0)
                else:
                    nc.scalar.activation(out=qk_sb[:, :w], in_=pt[:, :w],
                                         func=ACT_COPY, scale=1.0)
                rope(m, qk_sb, c0, w)

            def evac_v(m, e, pt, si, w):
                # seg si covers tokens [s0, s0+w); for si>0, s0 % 128 == 64.
                mv = m - 2 * HPC
                o1 = 64 if si else 0
                stg = tc_ev_pool.tile([128, 576], BF, tag="vstg", bufs=3,
                                      name="stg")
                if e == 0:
                    nc.scalar.activation(out=stg[:, o1:o1 + w],
                                         in_=pt[:, :w], func=ACT_ID,
                                         bias=bias_t[:, m:m + 1], scale=1.0)
                else:
                    nc.scalar.activation(out=stg[:, o1:o1 + w],
                                         in_=pt[:, :w], func=ACT_COPY,
                                         scale=1.0)
                # transposes are DEFERRED one m-block (returned as a closure)
                # so their input-ready waits never head-of-line block the
                # sync DMA FIFO in front of weight/hs streams.
                # split the 4 transposes across both HWDGE rings so neither
                # the weight stream (sync) nor the evac/exp queue (scalar)
                # eats the full burst
                if si == 0:
                    def tp():
                        for jt in range(4):
                            nc.sync.dma_start_transpose(
                                out=v_sb[mv][:, jt, :],
                                in_=stg[:, 128 * jt:128 * (jt + 1)])
                else:
                    b = si
                    nc.vector.tensor_copy(vTb[b][mv][:, 64:128],
                                          stg[:, 64:128])
                    if b + 1 <= 3 and w == 512:
                        nc.vector.tensor_copy(vTb[b + 1][mv][:, 0:64],
                                              stg[:, 576 - 64:576])

                    def tp():
                        nc.sync.dma_start_transpose(
                            out=v_sb[mv][:, 4 * b, :], in_=vTb[b][mv][:])
                        for i in range(3):
                            nc.sync.dma_start_transpose(
                                out=v_sb[mv][:, 4 * b + 1 + i, :],
                                in_=stg[:, 128 * (i + 1):128 * (i + 2)])
                return tp

            with tc.tile_pool(name="hsp", bufs=2) as hs_pool, \
                 tc.tile_pool(name="wp", bufs=2) as tc_w_pool, \
                 tc.tile_pool(name="evp", bufs=2) as tc_ev_pool:

                # seg0 streams first so the PE starts ASAP; constants load
                # behind them.
                # quarter-granularity first weight load: the very first
                # matmul only needs kt 0..7 resident
                wt0 = tc_w_pool.tile([128, NKT, 128], BF, tag="w", name="wt")
                b0 = (0 * NM + 8) * NKT * 128
                for q in range(4):
                    nc.sync.dma_start(
                        out=wt0[:, 8 * q:8 * (q + 1), :],
                        in_=w_d[:, b0 + q * 1024:b0 + (q + 1) * 1024])
                    if q == 0:
                        hst0 = hs_pool.tile([128, NKT, 512], BF, tag="hs",
                                            name="hst")
                        nc.sync.dma_start(out=hst0[:, 0:8, :],
                                          in_=hs3[:, 0:8, 0:512])
                        nc.sync.dma_start(out=hs_sliv[:, 0:8, :],
                                          in_=hs3[:, 0:8, 512:576])
                wpre = {(0, 8): wt0}
                for kg in range(8, NKT, 8):
                    nc.sync.dma_start(out=hst0[:, kg:kg + 8, :],
                                      in_=hs3[:, kg:kg + 8, 0:512])
                nc.sync.dma_start(out=hs_sliv[:, 8:, :],
                                  in_=hs3[:, 8:, 512:576])
                nc.sync.dma_start(out=bias_t[:], in_=bias_d[:])
                nc.sync.dma_start(out=rm_t[:], in_=rm_d[:])
                wpre[(0, 9)] = load_w(0, 9)
                nc.sync.dma_start(out=cos_t[:, :576], in_=cos_d[:, :576])
                nc.sync.dma_start(out=sin_t[:, :576], in_=sin_d[:, :576])
                nc.sync.dma_start(out=ones_t[:], in_=ones_d[:])
                nc.sync.dma_start(out=onesr_t[:], in_=onesr_d[:])
                nc.sync.dma_start(out=mask_t[:], in_=mask_d[:])
                nc.sync.dma_start(out=cos_t[:, 576:], in_=cos_d[:, 576:])
                nc.sync.dma_start(out=sin_t[:, 576:], in_=sin_d[:, 576:])
                # warm the exp table set early (one-element activation)
                exp_warm = tc_ev_pool.tile([1, 1], F32, tag="ew", bufs=1,
                                           name="exp_warm")
                nc.scalar.activation(out=exp_warm[:], in_=bias_t[0:1, 0:1],
                                     func=ACT_EXP, scale=1.0)

                hs_cur = hst0
                active = []          # attention chunks with remaining work
                pend_tp = []         # deferred v-transpose closures
                wd_loaded = []

                def pump_slot(n):
                    while n > 0 and active:
                        used = active[0].pump_some(n)
                        if active[0].done():
                            active.pop(0)
                        if used == 0 and not active:
                            break
                        n -= used

                for si, (s0, s1, e) in enumerate(SEGS):
                    sw = s1 - s0 if si else 512
                    hst = hs_cur
                    if si + 1 < len(SEGS):
                        n0, n1, _ = SEGS[si + 1]
                        hs_cur = hs_pool.tile([128, NKT, 512], BF, tag="hs",
                                              name="hst")
                    if si >= 1:
                        active.append(AttnChunk(si - 1))

                    for mi, m in enumerate(M_ORDER):
                        wt = wpre.pop((e, m), None)
                        if wt is None:
                            wt = load_w(e, m)
                        # prefetch 1 m-block ahead (same or next segment);
                        # the weight stream goes on the ring BEFORE the
                        # deferred transposes so it lands one burst earlier
                        pf = mi + 1
                        if pf < len(M_ORDER):
                            key = (e, M_ORDER[pf])
                            if key not in wpre:
                                wpre[key] = load_w(*key)
                        elif si + 1 < len(SEGS):
                            key = (SEGS[si + 1][2], M_ORDER[pf - len(M_ORDER)])
                            if key not in wpre:
                                wpre[key] = load_w(*key)
                        # flush deferred transposes from the previous block
                        for tp in pend_tp:
                            tp()
                        pend_tp = []
                        # spread next segment's hs prefetch into the q/k
                        # m-blocks (mi>=4): the v blocks' transposes already
                        # load the sync ring
                        if si + 1 < len(SEGS) and mi in (5, 7, 9, 11):
                            kg = 8 * ((mi - 5) // 2)
                            nc.sync.dma_start(
                                out=hs_cur[:, kg:kg + 8, :n1 - n0],
                                in_=hs3[:, kg:kg + 8, n0:n1])

                        ptA = pt_pool.tile([128, 512], F32, tag="pt",
                                           name="ptA")
                        ptB = None
                        if si == 0:
                            # the ap pool is idle during seg0 (no attention)
                            ptB = ap_pool.tile([128, 512], F32, tag="ap",
                                               name="ptB")
                        for half in range(2):
                            k0 = 16 * half
                            for kt in range(k0, k0 + 16):
                                nc.tensor.matmul(ptA[:, :sw], wt[:, kt, :],
                                                 hst[:, kt, :sw],
                                                 start=(kt == 0),
                                                 stop=(kt == NKT - 1))
                                if ptB is not None:
                                    nc.tensor.matmul(
                                        ptB[:, :64], wt[:, kt, :],
                                        hs_sliv[:, kt, :],
                                        start=(kt == 0),
                                        stop=(kt == NKT - 1))
                            pump_slot(3)
                        if m < 2 * HPC:
                            evac_qk(m, e, ptA, sw, s0)
                            if ptB is not None:
                                qkb = tc_ev_pool.tile([128, 64], BF,
                                                      tag="qkb", bufs=2,
                                                      name="qkb")
                                nc.scalar.activation(
                                    out=qkb[:], in_=ptB[:, :64],
                                    func=ACT_ID,
                                    bias=bias_t[:, m:m + 1], scale=1.0)
                                rope(m, qkb, 512, 64)
                        else:
                            pend_tp.append(evac_v(m, e, ptA, si, sw))
                            if ptB is not None:
                                mv = m - 2 * HPC
                                nc.scalar.activation(
                                    out=vTb[1][mv][:, 0:64],
                                    in_=ptB[:, :64], func=ACT_ID,
                                    bias=bias_t[:, m:m + 1], scale=1.0)

                    for tp in pend_tp:
                        tp()
                    pend_tp = []

                    if si == 3:
                        # finish any remaining chunk-2 work before dense
                        while active:
                            active[0].pump_some(10**6)
                            active.pop(0)
                        # prefetch first dense weight groups (4-o tiles)
                        for og in range(2):
                            for ee in range(2):
                                wdt = wd_pool.tile([128, 4, HPC, 128], BF,
                                                   tag="wd", name="wdt")
                                base = (ee * 32 + 4 * og) * HPC * 128
                                nc.sync.dma_start(
                                    out=wdt[:],
                                    in_=wd_d[:, base:base + 4 * HPC * 128])
                                wd_loaded.append(((ee, og), wdt))

            # ---------------- dense phase ----------------
            # pass A: tokens 0..1536 (interleaved with attn chunk 3),
            # pass B: tokens 1536..2048 (needs chunk 3's attnT).
            DCH_A = [[(0, 512), (512, 576)], [(576, 1088), (1088, 1536)]]
            last_chunk = AttnChunk(3)
            wd_tiles = dict(wd_loaded)

            def load_wd(ee, og):
                t = wd_pool.tile([128, 4, HPC, 128], BF, tag="wd",
                                 name="wdt")
                base = (ee * 32 + 4 * og) * HPC * 128
                nc.sync.dma_start(
                    out=t[:], in_=wd_d[:, base:base + 4 * HPC * 128])
                return t

            def get_wd(ee, og):
                t = wd_tiles.get((ee, og))
                if t is None:
                    t = load_wd(ee, og)
                    wd_tiles[(ee, og)] = t
                return t

            with tc.tile_pool(name="oep", bufs=3) as oe_pool:
                nsub = 32
                usched = [len(last_chunk.units) // nsub +
                          (1 if i < len(last_chunk.units) % nsub else 0)
                          for i in range(nsub)]
                sub_i = 0
                ei = 0
                for og in range(8):
                    for ee in range(2):
                        get_wd(ee, og)
                    if og + 2 < 8:
                        for ee in range(2):
                            get_wd(ee, og + 2)
                    for oi in range(HPC):
                        o = 4 * og + oi
                        oe = oe_pool.tile([128, 1536], BF, tag="oe",
                                          name="oe")
                        for ee in range(2):
                            wdt = get_wd(ee, og)
                            pos = [pt_pool.tile([128, 512], F32, tag="pt",
                                                name="po")
                                   for _ in DCH_A[ee]]
                            for hh in range(HPC):
                                for ci, (t0, t1) in enumerate(DCH_A[ee]):
                                    nc.tensor.matmul(
                                        pos[ci][:, :t1 - t0],
                                        wdt[:, oi, hh, :],
                                        attnT[hh][:, t0:t1],
                                        start=(hh == 0),
                                        stop=(hh == HPC - 1))
                            for ci, (t0, t1) in enumerate(DCH_A[ee]):
                                if ei % 2 == 0:
                                    nc.scalar.activation(
                                        out=oe[:, t0:t1],
                                        in_=pos[ci][:, :t1 - t0],
                                        func=ACT_COPY, scale=1.0)
                                else:
                                    nc.vector.tensor_copy(
                                        oe[:, t0:t1], pos[ci][:, :t1 - t0])
                                ei += 1
                        nc.scalar.dma_start(
                            out=out_d[128 * o:128 * (o + 1), 0:1536],
                            in_=oe[:])
                        last_chunk.pump_some(usched[sub_i])
                        sub_i += 1
                last_chunk.pump_some(10**6)

                # pass B — expert-1 tiles were recycled by the ring; reload.
                # accumulators alternate between the pt and (now idle) ap
                # pools for 4-deep psum pipelining.
                wdB = {0: load_wd(1, 0), 1: load_wd(1, 1)}
                for og in range(8):
                    wdt = wdB.pop(og)
                    if og + 2 < 8:
                        wdB[og + 2] = load_wd(1, og + 2)
                    for oi in range(HPC):
                        o = 4 * og + oi
                        if oi % 2 == 0:
                            po = pt_pool.tile([128, 512], F32, tag="pt",
                                              name="poB")
                        else:
                            po = ap_pool.tile([128, 512], F32, tag="ap",
                                              name="poB")
                        for hh in range(HPC):
                            nc.tensor.matmul(po[:], wdt[:, oi, hh, :],
                                             attnT[hh][:, 1536:2048],
                                             start=(hh == 0),
                                             stop=(hh == HPC - 1))
                        oe2 = oe_pool.tile([128, 512], BF, tag="oe2",
                                           name="oe2")
                        if ei % 2 == 0:
                            nc.scalar.activation(out=oe2[:], in_=po[:],
                                                 func=ACT_COPY, scale=1.0)
                        else:
                            nc.vector.tensor_copy(oe2[:], po[:])
                        ei += 1
                        nc.sync.dma_start(
                            out=out_d[128 * o:128 * (o + 1), 1536:2048],
                            in_=oe2[:])

            ap_ctx.__exit__(None, None, None)
            pt_ctx.__exit__(None, None, None)
            ps_ctx.__exit__(None, None, None)
            atn_ctx.__exit__(None, None, None)
            wd_ctx.__exit__(None, None, None)
    nc.compile()
    return nc


def _prep_inputs(inputs):
    import ml_dtypes
    bf = ml_dtypes.bfloat16

    hs = np.asarray(inputs["hidden_states"], np.float32)
    cos = np.asarray(inputs["cos"], np.float32)
    sin = np.asarray(inputs["sin"], np.float32)
    vi = np.asarray(inputs["vision_indices"]).ravel()
    li = np.asarray(inputs["language_indices"]).ravel()
    assert vi.size == NV and np.array_equal(vi, np.arange(NV)) and \
        np.array_equal(li, np.arange(NV, S)), "unsupported index layout"

    # hs tiled [128, (kt, t)]
    hs_t = np.ascontiguousarray(
        hs.T.reshape(NKT, 128, S).transpose(1, 0, 2).reshape(128, NKT * S)
    ).astype(bf)

    cos_t = np.ascontiguousarray(cos.T).astype(bf)
    sin_t = np.ascontiguousarray(sin.T).astype(bf)

    rm = np.zeros((D, D), np.float32)
    for d in range(64):
        rm[d + 64, d] = -1.0
        rm[d, d + 64] = 1.0
    rm = rm.astype(bf)

    # mask[p, q'] = 1 iff p <= q' within a 128 diagonal block
    p = np.arange(128)[:, None]
    q = np.arange(128)[None, :]
    mask = (p <= q).astype(np.float32).astype(bf)

    ones = np.ones((128, 1), np.float32).astype(bf)

    b = np.asarray(inputs["vision_qkv_b"], np.float32)
    Wqkv = np.stack([np.asarray(inputs["vision_qkv_w"], np.float32),
                     np.asarray(inputs["lang_qkv_w"], np.float32)])
    Wd = np.stack([np.asarray(inputs["vision_dense_w"], np.float32),
                   np.asarray(inputs["lang_dense_w"], np.float32)])

    in_maps = []
    for rr in range(NCORES):
        q0 = 512 * rr
        cols = np.r_[q0:q0 + 512, HID + q0:HID + q0 + 512,
                     2 * HID + q0:2 * HID + q0 + 512]
        # w tiled [128, (e, m, kt, c)]
        wc = Wqkv[:, :, cols]                                # [2, 4096, 1536]
        w_t = np.ascontiguousarray(
            wc.reshape(2, NKT, 128, NM, 128)
              .transpose(2, 0, 3, 1, 4)
              .reshape(128, 2 * NM * NKT * 128)).astype(bf)
        # wd tiled [128, (e, o, hh, c)]
        wdc = Wd[:, q0:q0 + 512, :]                          # [2, 512, 4096]
        wd_t = np.ascontiguousarray(
            wdc.reshape(2, HPC, 128, 32, 128)
               .transpose(2, 0, 3, 1, 4)
               .reshape(128, 2 * 32 * HPC * 128)).astype(bf)
        bias_t = np.ascontiguousarray(
            b[cols].reshape(NM, 128).T).astype(np.float32)
        in_maps.append({
            "hs": hs_t, "w": w_t, "wd": wd_t,
            "cos": cos_t, "sin": sin_t, "mask": mask, "rm": rm,
            "ones": ones, "onesr": np.ones((1, 128), np.float32).astype(bf),
            "bias": bias_t,
        })
    return in_maps


def kernel(**inputs):
    if "nc" not in _CACHE:
        _CACHE["nc"] = _build()
    nc = _CACHE["nc"]
    in_maps = _prep_inputs(inputs)
    res = run_bass_kernel_spmd(nc, in_maps, list(range(NCORES)),
                               **_CACHE.get("run_kwargs", {}))
    _CACHE["last_results"] = res
    out = np.zeros((HID, S), np.float32)
    for r in range(NCORES):
        out += res.results[r]["outT"].astype(np.float32)
    return np.ascontiguousarray(out.T)
